# revision 13
# baseline (speedup 1.0000x reference)
"""Causal multi-head self-attention (RoPE) Trainium2 Bass kernel.

Problem: B=4, T=2048, d_model=1024, 16 heads, d_k=64, causal, RoPE,
fp32 I/O.  Sharding: batch (4-way) x head-group (2-way) over 8 cores.
Core c handles batch c//2 and heads [8*(c%2) .. 8*(c%2)+8).

Everything on device runs in the "transposed" domain to avoid on-chip
transposes entirely:
  QT/KT:  [head_dim, T]   (head dim on partitions), bf16
  V:      [T, head_dim]   (k positions on partitions), bf16 + ones col
  scores: S^T [k, q] = KT_tile^T @ QT  (per head), fp32 PSUM
  E = exp(S^T/8) in bf16; causal masking of the diagonal blocks via
  gpsimd affine_select (zero out k>q entries after exp)
  PV: H^T[d, q] = [V|ones]^T @ E  -> heads rows 0:64 + sums row 64
  normalize via fast reciprocal of the sums row + gpsimd
  partition_broadcast + DVE multiplies
  out-proj: y[t, e] = H^T_tile^T @ WoT, accumulated over head pairs

All matmuls run in bf16 (1 cycle/row at any moving size); accumulation
is fp32 in PSUM.  RoPE is applied as rot = cos (.) pre + sin (.)
(P2 @ pre), where the head dim has been host-permuted to rotate-half
layout and P2 is the fixed swap-negate permutation folded into a
128x128 matmul.

The k-loop and projection loops are software-pipelined by one stage so
the PE never stalls on ACT (exp / psum evacuation) latency.
"""

import numpy as np
from contextlib import ExitStack

import concourse.bass as bass
from concourse import bacc
import concourse.tile as tile
import concourse.mybir as mybir
import ml_dtypes
from concourse.bass_utils import run_bass_kernel_spmd

F32 = mybir.dt.float32
BF16 = mybir.dt.bfloat16
AF = mybir.ActivationFunctionType
ALU = mybir.AluOpType

D_MODEL = 1024
NUM_HEADS = 16
THETA = 10000.0
B, T = 4, 2048
N_CORES = 8
PAIRS = 4             # head pairs per core (8 heads)
QC = 512              # q-chunk width
NQC = T // QC
KT = 128              # k-tile height
NKT = T // KT
XC = 256              # xt streaming chunk width (phase A)
NXC = T // XC
DMT = D_MODEL // 128  # 8 d_model k-tiles

_CACHE = {}


def _build_nc():
    nc = bacc.Bacc(None, target_bir_lowering=False)

    xt = nc.dram_tensor("xt", [D_MODEL, T], BF16, kind="ExternalInput")
    wq = nc.dram_tensor("wq", [D_MODEL, 512], BF16, kind="ExternalInput")
    wk = nc.dram_tensor("wk", [D_MODEL, 512], BF16, kind="ExternalInput")
    wv = nc.dram_tensor("wv", [D_MODEL, 512], BF16, kind="ExternalInput")
    wo = nc.dram_tensor("wo", [512, D_MODEL], BF16, kind="ExternalInput")
    cosd = nc.dram_tensor("cos", [128, T], BF16, kind="ExternalInput")
    sind = nc.dram_tensor("sin", [128, T], BF16, kind="ExternalInput")
    p2t = nc.dram_tensor("p2t", [128, 128], BF16, kind="ExternalInput")
    y = nc.dram_tensor("y", [T, D_MODEL], F32, kind="ExternalOutput")

    with tile.TileContext(nc) as tc, ExitStack() as ctx:
        constp = ctx.enter_context(tc.tile_pool(name="const", bufs=1))
        qkv_stack = ExitStack()
        qkp = qkv_stack.enter_context(tc.tile_pool(name="qk", bufs=1))
        vp = qkv_stack.enter_context(tc.tile_pool(name="v", bufs=1))

        cos_sb = constp.tile([128, T], BF16)
        sin_sb = constp.tile([128, T], BF16)
        p2_sb = constp.tile([128, 128], BF16)
        # wo lives in a long-lived pool so it can be prefetched in phase B
        wo_sb = constp.tile([128, PAIRS, D_MODEL], BF16)

        qt_t = [qkp.tile([128, T], BF16, name=f"qt{p}", tag=f"qt{p}")
                for p in range(PAIRS)]
        kt_t = [qkp.tile([128, T], BF16, name=f"kt{p}", tag=f"kt{p}")
                for p in range(PAIRS)]
        # V with interleaved ones cols: per k-tile, per pair:
        # [V_h0(64) | 1 | V_h1(64) | 1] = 130 cols
        v_sb = vp.tile([128, NKT, PAIRS * 130], BF16)
        v5 = v_sb.rearrange("p t (pr x) -> p t pr x", pr=PAIRS)
        v6 = v5.rearrange("p t pr (hl c) -> p t pr hl c", hl=2)
        nc.vector.memset(v6[:, :, :, :, 64:65], 1.0)

        # x and wv stay resident through phase B: the V projections of
        # pairs 1-3 are interleaved into the attention windows (which are
        # Activation-bound) as pure-PE filler work
        xp = qkv_stack.enter_context(tc.tile_pool(name="xp", bufs=1))
        wvp = qkv_stack.enter_context(tc.tile_pool(name="wvp", bufs=1))
        x_sb = xp.tile([128, DMT, T], BF16)
        wv_sb = wvp.tile([128, DMT, 512], BF16)
        psX = qkv_stack.enter_context(tc.tile_pool(name="psX", bufs=2,
                                                   space="PSUM"))
        yst = qkv_stack.enter_context(tc.tile_pool(name="yst", bufs=3))
        hp = qkv_stack.enter_context(
            tc.tile_pool(name="hp", bufs=1, side="right"))
        h_t = [hp.tile([128, T], BF16, name=f"h{p}", tag=f"h{p}")
               for p in range(PAIRS)]

        def v_unit(pp, kti):
            ps_x = psX.tile([128, 512], F32, tag="x", name="ps_v")
            ps_v = ps_x[:, 0:128]
            for dmt in range(DMT):
                nc.tensor.matmul(
                    ps_v, x_sb[:, dmt, kti * 128:(kti + 1) * 128],
                    wv_sb[:, dmt, pp * 128:(pp + 1) * 128],
                    start=(dmt == 0), stop=(dmt == DMT - 1))
            src = ps_v.rearrange("p (hl c) -> p hl c", hl=2)
            nc.gpsimd.tensor_copy(v6[:, kti, pp, :, 0:64], src)

        # ---------------- Phase A: projections + RoPE ----------------
        with tc.tile_pool(name="wqkv", bufs=1) as wp, \
             tc.tile_pool(name="sA", bufs=4) as sA, \
             tc.tile_pool(name="psA", bufs=2, space="PSUM") as psA:
            wq_sb = wp.tile([128, DMT, 512], BF16)
            wk_sb = wp.tile([128, DMT, 512], BF16)
            xt_r = xt.rearrange("(a p) t -> p a t", p=128)
            wq_r = wq.rearrange("(a p) m -> p a m", p=128)
            wk_r = wk.rearrange("(a p) m -> p a m", p=128)
            wv_r = wv.rearrange("(a p) m -> p a m", p=128)
            # priority order: interleave wq + first x chunk per-dmt so the
            # first matmul group can start as soon as its slices land;
            # everything else trickles in behind them.
            for dmt in range(DMT):
                nc.sync.dma_start(wq_sb[:, dmt, :], wq_r[:, dmt, :])
                nc.sync.dma_start(x_sb[:, dmt, 0:XC], xt_r[:, dmt, 0:XC])
            # rope constants are needed ~3us in, right after the first
            # projection group
            nc.sync.dma_start(p2_sb, p2t[:])
            nc.sync.dma_start(cos_sb, cosd[:])
            nc.sync.dma_start(sin_sb, sind[:])
            for dmt in range(DMT):
                nc.sync.dma_start(wk_sb[:, dmt, :], wk_r[:, dmt, :])
            nc.sync.dma_start(x_sb[:, :, XC:2 * XC], xt_r[:, :, XC:2 * XC])
            # wv is needed by the first V unit at the start of chunk 1
            for dmt in range(DMT):
                nc.sync.dma_start(wv_sb[:, dmt, :], wv_r[:, dmt, :])
            for c in range(2, NXC):
                cs = slice(c * XC, (c + 1) * XC)
                nc.sync.dma_start(x_sb[:, :, cs], xt_r[:, :, cs])

            def rope_stage(st):
                pre, dst, cs = st
                ps_a = psA.tile([128, XC], F32, tag="alt", name="ps_a")
                nc.tensor.matmul(ps_a, p2_sb, pre, start=True, stop=True)
                t1 = sA.tile([128, XC], F32, tag="t1", name="t1")
                nc.vector.tensor_mul(t1, sin_sb[:, cs], ps_a)
                t2 = sA.tile([128, XC], F32, tag="t2", name="t2")
                nc.gpsimd.tensor_mul(t2, cos_sb[:, cs], pre)
                nc.vector.tensor_add(dst[:, cs], t1, t2)

            for c in range(NXC):
                cs = slice(c * XC, (c + 1) * XC)
                pend = None
                if c > 0:
                    # pair-0 V for the previous chunk (deferred one chunk
                    # so the wv DMA has time to land)
                    v_unit(0, 2 * (c - 1))
                    v_unit(0, 2 * (c - 1) + 1)
                for (w_sb, dsts) in ((wq_sb, qt_t), (wk_sb, kt_t)):
                    for p in range(PAIRS):
                        ps = psA.tile([128, XC], F32, tag="proj", name="ps")
                        for dmt in range(DMT):
                            nc.tensor.matmul(
                                ps, w_sb[:, dmt, p * 128:(p + 1) * 128],
                                x_sb[:, dmt, cs],
                                start=(dmt == 0), stop=(dmt == DMT - 1))
                        pre = sA.tile([128, XC], BF16, tag="pre", name="pre")
                        nc.scalar.copy(pre, ps)
                        if pend is not None:
                            rope_stage(pend)
                        pend = (pre, dsts[p], cs)

                rope_stage(pend)
            v_unit(0, 14)
            v_unit(0, 15)

        # ---------------- Phase B: attention ----------------
        with tc.tile_pool(name="ep", bufs=6) as ep, \
             tc.tile_pool(name="h1p", bufs=2) as h1p, \
             tc.tile_pool(name="sB", bufs=2) as sB, \
             tc.tile_pool(name="shp", bufs=2) as shp, \
             tc.tile_pool(name="rbp", bufs=2) as rbp, \
             tc.tile_pool(name="psS", bufs=2, space="PSUM") as psS, \
             tc.tile_pool(name="psH", bufs=1, space="PSUM") as psH:

            # prefetch the out-projection weights while the PE chews on
            # attention — the DMA engines are nearly idle in phase B
            nc.sync.dma_start(
                wo_sb, wo.rearrange("(a p) m -> p a m", p=128))

            stC = {"ysb": None}

            def c_unit(tt, ec):
                ts_ = slice(tt * 128, (tt + 1) * 128)
                es = slice(ec * 512, (ec + 1) * 512)
                if ec == 0:
                    stC["ysb"] = yst.tile([128, 2, 512], F32, tag="y",
                                          name="y_sb")
                y_sb = stC["ysb"]
                ps_y = psX.tile([128, 512], F32, tag="x", name="ps_y")
                for p in range(PAIRS):
                    nc.tensor.matmul(ps_y, h_t[p][:, ts_], wo_sb[:, p, es],
                                     start=(p == 0), stop=(p == PAIRS - 1))
                # alternate evacuation between ACT and DVE
                if ec == 0:
                    nc.scalar.copy(y_sb[:, 0, :], ps_y)
                else:
                    nc.vector.tensor_copy(y_sb[:, 1, :], ps_y)
                    # one full-row DMA per t-tile
                    nc.sync.dma_start(y[ts_, :], y_sb)

            def emit_pv(st):
                p, ps_h, e_sb, c0, k, last = st
                nc.tensor.matmul(ps_h[0:65, 0, c0:],
                                 v_sb[:, k, 130 * p:130 * p + 65],
                                 e_sb[:, 0, c0:],
                                 start=(k == 0), stop=last)
                nc.tensor.matmul(ps_h[0:65, 1, c0:],
                                 v_sb[:, k, 130 * p + 65:130 * p + 130],
                                 e_sb[:, 1, c0:],
                                 start=(k == 0), stop=last)

            def emit_norm_head(p, ps_h, qc, h1):
                # evacuate H + sums to SBUF in one copy: ps_h (the single
                # PSUM PV buffer) frees after just this op, so the next
                # q-chunk's first PV is not gated on the whole chain
                s_h = shp.tile([65, 2, 512], F32, tag="sh", name="s_h")
                nc.vector.tensor_copy(s_h, ps_h[0:65, :, :])
                # the custom-DVE reciprocal misreads rows at non-zero base
                # partitions on hardware — stage the sums row at partition 0
                s1 = sB.tile([1, 2, 512], F32, tag="s1", name="s1")
                nc.vector.tensor_copy(s1, s_h[64:65, :, :])
                r1 = sB.tile([1, 2, 512], F32, tag="r1", name="r1")
                nc.vector.reciprocal_approx_fast(out=r1, in_=s1)
                r64 = rbp.tile([64, 2, 512], F32, tag="r64", name="r64")
                nc.gpsimd.partition_broadcast(r64, r1)
                return (p, qc, h1, s_h, r64)

            def emit_norm_tail(pend):
                # deferred: the DVE queue only sees these multiplies once
                # the Pool broadcast has surely landed (strict FIFOs — a
                # waiting instruction blocks everything behind it)
                p, qc, h1, s_h, r64 = pend
                qs = slice(qc * QC, (qc + 1) * QC)
                nc.vector.tensor_mul(h_t[p][0:64, qs], s_h[0:64, 0, :],
                                     r64[0:64, 0, :])
                nc.vector.tensor_mul(h1[0:64, :], s_h[0:64, 1, :],
                                     r64[0:64, 1, :])
                # odd head rows into partitions 64:128 of the pair tile
                nc.sync.dma_start(h_t[p][64:128, qs], h1[0:64, :])

            pend_pv = []
            pend_norm = None
            feed = []
            for p in range(PAIRS):
                if p < PAIRS - 1:
                    feed = [lambda pp=p + 1, kti=kti: v_unit(pp, kti)
                            for kti in range(NKT)]
                qt, kt = qt_t[p], kt_t[p]
                for qc in range(NQC):
                    nk = 4 * (qc + 1)
                    ps_h = psH.tile([128, 2, 512], F32, tag="pv", name="ps_h")
                    h1 = h1p.tile([64, QC], BF16, tag="h1", name="h1")
                    for k in range(nk):
                        m = k - 4 * qc
                        c0 = 128 * m if m >= 0 else 0
                        qs = slice(qc * QC + c0, (qc + 1) * QC)
                        ks = slice(k * KT, (k + 1) * KT)
                        ps_s = psS.tile([128, 2, 512], F32, tag="s",
                                        name="ps_s")
                        diag = m >= 0
                        nc.tensor.matmul(ps_s[:, 0, c0:], kt[0:64, ks],
                                         qt[0:64, qs], start=True,
                                         stop=True)
                        nc.tensor.matmul(ps_s[:, 1, c0:], kt[64:128, ks],
                                         qt[64:128, qs], start=True,
                                         stop=True)
                        e_sb = ep.tile([128, 2, 512], BF16, tag="e",
                                       name="e_sb")
                        nc.scalar.activation(e_sb[:, :, c0:], ps_s[:, :, c0:],
                                             AF.Exp, scale=0.125)
                        if diag:
                            # zero the strictly-upper-tri (k>q) entries of
                            # the diagonal block after exp
                            nc.gpsimd.affine_select(
                                e_sb[:, :, c0:c0 + 128],
                                e_sb[:, :, c0:c0 + 128],
                                pattern=[[0, 2], [1, 128]],
                                compare_op=ALU.is_ge,
                                fill=0.0, base=0, channel_multiplier=-1)
                        if len(pend_pv) >= 2:
                            emit_pv(pend_pv.pop(0))
                        pend_pv.append((p, ps_h, e_sb, c0, k, k == nk - 1))
                        # fire the deferred norm tail only once the Pool
                        # broadcast had time to land
                        if pend_norm is not None and k >= 3:
                            emit_norm_tail(pend_norm)
                            pend_norm = None
                        # PE filler: V projections of the next pair (or,
                        # for the last pair, out-projection tiles; k>=3 so
                        # they follow this q-chunk's norm tail)
                        if p < PAIRS - 1:
                            if feed and k % 2 == 1:
                                feed.pop(0)()
                        elif k >= 3:
                            if feed:
                                feed.pop(0)()
                            if feed and k % 2 == 0:
                                feed.pop(0)()
                    # cover the last exp's latency, then drain the PVs and
                    # normalize this q-chunk
                    if feed and p < PAIRS - 1:
                        feed.pop(0)()
                    while pend_pv:
                        emit_pv(pend_pv.pop(0))
                    if pend_norm is not None:
                        emit_norm_tail(pend_norm)
                    pend_norm = emit_norm_head(p, ps_h, qc, h1)
                    if p == PAIRS - 1:
                        for tt in range(4 * qc, 4 * qc + 4):
                            feed.append(lambda tt=tt: c_unit(tt, 0))
                            feed.append(lambda tt=tt: c_unit(tt, 1))
                if p < PAIRS - 1:
                    # the next pair's attention needs its V complete
                    while feed:
                        feed.pop(0)()
            emit_norm_tail(pend_norm)
            # tail: remaining out-projection tiles
            while feed:
                feed.pop(0)()

        qkv_stack.close()

    nc.compile()
    return nc


def _host_prep(in_features, token_positions, Wq, Wk, Wv, Wo):
    """Shard + pre-transpose + bf16-cast inputs for the 8 cores."""
    x = np.asarray(in_features, dtype=np.float32)
    pos = np.asarray(token_positions)
    Wq = np.asarray(Wq, dtype=np.float32)
    Wk = np.asarray(Wk, dtype=np.float32)
    Wv = np.asarray(Wv, dtype=np.float32)
    Wo = np.asarray(Wo, dtype=np.float32)

    # rotate-half permutation of each head's 64 dims: evens then odds
    perm = np.concatenate([np.arange(0, 64, 2), np.arange(1, 64, 2)])
    full_perm = (np.arange(NUM_HEADS)[:, None] * 64 + perm[None, :]).reshape(-1)
    Wq_p = Wq[full_perm, :]   # permute output rows (head dims)
    Wk_p = Wk[full_perm, :]

    # P2: alt = P2 @ pre (per 64-block: alt[i] = -pre[32+i], alt[32+i]=pre[i])
    p2 = np.zeros((128, 128), np.float32)
    for blk in (0, 64):
        for i in range(32):
            p2[blk + i, blk + 32 + i] = -1.0
            p2[blk + 32 + i, blk + i] = 1.0
    p2t = np.ascontiguousarray(p2.T).astype(ml_dtypes.bfloat16)

    inv_freq = 1.0 / (THETA ** (np.arange(32, dtype=np.float64) * 2.0 / 64))

    bf = ml_dtypes.bfloat16
    in_maps = []
    for core in range(N_CORES):
        b = core // 2
        g = core % 2
        hs = slice(g * 512, (g + 1) * 512)   # head-dim slice of d_model

        ang = pos[b].astype(np.float64)[None, :] * inv_freq[:, None]  # [32,T]
        cos64 = np.cos(ang).astype(np.float32)
        sin64 = np.sin(ang).astype(np.float32)
        cos128 = np.tile(np.concatenate([cos64, cos64], 0), (2, 1))   # [128,T]
        sin128 = np.tile(np.concatenate([sin64, sin64], 0), (2, 1))

        in_maps.append({
            "xt": np.ascontiguousarray(x[b].T).astype(bf),
            "wq": np.ascontiguousarray(Wq_p[hs, :].T).astype(bf),
            "wk": np.ascontiguousarray(Wk_p[hs, :].T).astype(bf),
            "wv": np.ascontiguousarray(Wv[hs, :].T).astype(bf),
            "wo": np.ascontiguousarray(Wo[:, hs].T).astype(bf),
            "cos": np.ascontiguousarray(cos128).astype(bf),
            "sin": np.ascontiguousarray(sin128).astype(bf),
            "p2t": p2t,
        })
    return in_maps


def kernel(**inputs):
    if "nc" not in _CACHE:
        _CACHE["nc"] = _build_nc()
    nc = _CACHE["nc"]
    in_maps = _host_prep(**inputs)
    res = run_bass_kernel_spmd(nc, in_maps, core_ids=list(range(N_CORES)))
    out = np.zeros((B, T, D_MODEL), np.float32)
    for core in range(N_CORES):
        out[core // 2] += res.results[core]["y"]
    return out


# revision 23
# speedup vs baseline: 1.0584x; 1.0584x over previous
"""Causal multi-head self-attention (RoPE) Trainium2 Bass kernel.

Problem: B=4, T=2048, d_model=1024, 16 heads, d_k=64, causal, RoPE,
fp32 I/O.  Sharding: batch (4-way) x head-group (2-way) over 8 cores.
Core c handles batch c//2 and heads [8*(c%2) .. 8*(c%2)+8).

Everything on device runs in the "transposed" domain to avoid on-chip
transposes entirely:
  QT/KT:  [head_dim, T]   (head dim on partitions), bf16
  V:      [T, head_dim]   (k positions on partitions), bf16 + ones col
  scores: S^T [k, q] = KT_tile^T @ QT  (per head), fp32 PSUM
  E = exp(S^T/8) in bf16; causal masking of the diagonal blocks via
  gpsimd affine_select (zero out k>q entries after exp)
  PV: H^T[d, q] = [V|ones]^T @ E  -> heads rows 0:64 + sums row 64
  normalize via fast reciprocal of the sums row + gpsimd
  partition_broadcast + DVE multiplies
  out-proj: y[t, e] = H^T_tile^T @ WoT, accumulated over head pairs

All matmuls run in bf16 (1 cycle/row at any moving size); accumulation
is fp32 in PSUM.  RoPE is applied as rot = cos (.) pre + sin (.)
(P2 @ pre), where the head dim has been host-permuted to rotate-half
layout and P2 is the fixed swap-negate permutation folded into a
128x128 matmul.

The k-loop and projection loops are software-pipelined by one stage so
the PE never stalls on ACT (exp / psum evacuation) latency.
"""

import numpy as np
from contextlib import ExitStack

import concourse.bass as bass
from concourse import bacc
import concourse.tile as tile
import concourse.mybir as mybir
import ml_dtypes
from concourse.bass_utils import run_bass_kernel_spmd

F32 = mybir.dt.float32
BF16 = mybir.dt.bfloat16
AF = mybir.ActivationFunctionType
ALU = mybir.AluOpType

D_MODEL = 1024
NUM_HEADS = 16
THETA = 10000.0
B, T = 4, 2048
N_CORES = 8
PAIRS = 4             # head pairs per core (8 heads)
QC = 512              # q-chunk width
NQC = T // QC
KT = 128              # k-tile height
NKT = T // KT
XC = 256              # xt streaming chunk width (phase A)
NXC = T // XC
DMT = D_MODEL // 128  # 8 d_model k-tiles

_CACHE = {}


def _build_nc():
    nc = bacc.Bacc(None, target_bir_lowering=False)

    xt = nc.dram_tensor("xt", [D_MODEL, T], BF16, kind="ExternalInput")
    wq = nc.dram_tensor("wq", [D_MODEL, 512], BF16, kind="ExternalInput")
    wk = nc.dram_tensor("wk", [D_MODEL, 512], BF16, kind="ExternalInput")
    wv = nc.dram_tensor("wv", [D_MODEL, 512], BF16, kind="ExternalInput")
    wo = nc.dram_tensor("wo", [512, D_MODEL], BF16, kind="ExternalInput")
    cosd = nc.dram_tensor("cos", [128, T], BF16, kind="ExternalInput")
    sind = nc.dram_tensor("sin", [128, T], BF16, kind="ExternalInput")
    p2t = nc.dram_tensor("p2t", [128, 128], BF16, kind="ExternalInput")
    y = nc.dram_tensor("y", [T, D_MODEL], F32, kind="ExternalOutput")

    with tile.TileContext(nc) as tc, ExitStack() as ctx:
        constp = ctx.enter_context(tc.tile_pool(name="const", bufs=1))
        qkv_stack = ExitStack()
        qkp = qkv_stack.enter_context(tc.tile_pool(name="qk", bufs=1))
        vp = qkv_stack.enter_context(tc.tile_pool(name="v", bufs=1))

        cos_sb = constp.tile([128, T], BF16)
        sin_sb = constp.tile([128, T], BF16)
        p2_sb = constp.tile([128, 128], BF16)
        # wo lives in a long-lived pool so it can be prefetched in phase B
        wo_sb = constp.tile([128, PAIRS, D_MODEL], BF16)
        # pair-3 odd-head rows of wo staged at partitions 0:64 so the final
        # out-proj tiles can read the h1 tile directly (skipping the h1->h_t
        # DMA on the tail critical path)
        wo2_sb = constp.tile([64, D_MODEL], BF16)

        qt_t = [qkp.tile([128, T], BF16, name=f"qt{p}", tag=f"qt{p}")
                for p in range(PAIRS)]
        kt_t = [qkp.tile([128, T], BF16, name=f"kt{p}", tag=f"kt{p}")
                for p in range(PAIRS)]
        # V with interleaved ones cols: per k-tile, per pair:
        # [V_h0(64) | 1 | V_h1(64) | 1] = 130 cols
        v_sb = vp.tile([128, NKT, PAIRS * 130], BF16)
        v5 = v_sb.rearrange("p t (pr x) -> p t pr x", pr=PAIRS)
        v6 = v5.rearrange("p t pr (hl c) -> p t pr hl c", hl=2)
        nc.vector.memset(v6[:, :, :, :, 64:65], 1.0)

        # x and wv stay resident through phase B: the V projections of
        # pairs 1-3 are interleaved into the attention windows (which are
        # Activation-bound) as pure-PE filler work
        xp = qkv_stack.enter_context(tc.tile_pool(name="xp", bufs=1))
        wvp = qkv_stack.enter_context(tc.tile_pool(name="wvp", bufs=1))
        x_sb = xp.tile([128, DMT, T], BF16)
        wv_sb = wvp.tile([128, DMT, 512], BF16)
        psX = qkv_stack.enter_context(tc.tile_pool(name="psX", bufs=2,
                                                   space="PSUM"))
        yst = qkv_stack.enter_context(tc.tile_pool(name="yst", bufs=3))
        hp = qkv_stack.enter_context(
            tc.tile_pool(name="hp", bufs=1, side="right"))
        h_t = [hp.tile([128, T], BF16, name=f"h{p}", tag=f"h{p}")
               for p in range(PAIRS)]

        def v_unit(pp, kti):
            ps_x = psX.tile([128, 512], F32, tag="x", name="ps_v")
            ps_v = ps_x[:, 0:128]
            for dmt in range(DMT):
                nc.tensor.matmul(
                    ps_v, x_sb[:, dmt, kti * 128:(kti + 1) * 128],
                    wv_sb[:, dmt, pp * 128:(pp + 1) * 128],
                    start=(dmt == 0), stop=(dmt == DMT - 1))
            src = ps_v.rearrange("p (hl c) -> p hl c", hl=2)
            nc.vector.tensor_copy(v6[:, kti, pp, :, 0:64], src)

        # ---------------- Phase A: projections + RoPE ----------------
        with tc.tile_pool(name="wqkv", bufs=1) as wp, \
             tc.tile_pool(name="sA", bufs=5) as sA, \
             tc.tile_pool(name="psA", bufs=3, space="PSUM") as psA:
            wq_sb = wp.tile([128, DMT, 512], BF16)
            wk_sb = wp.tile([128, DMT, 512], BF16)
            xt_r = xt.rearrange("(a p) t -> p a t", p=128)
            wq_r = wq.rearrange("(a p) m -> p a m", p=128)
            wk_r = wk.rearrange("(a p) m -> p a m", p=128)
            wv_r = wv.rearrange("(a p) m -> p a m", p=128)
            # priority order: interleave wq + first x chunk per-dmt so the
            # first matmul group can start as soon as its slices land;
            # everything else trickles in behind them.
            # consolidated prefix: every DMA pays ~625ns of HWDGE, so
            # fewer, larger transfers get wk/cos/sin on chip sooner
            nc.sync.dma_start(wq_sb[:, 0:4, :], wq_r[:, 0:4, :])
            nc.sync.dma_start(x_sb[:, 0:4, 0:XC], xt_r[:, 0:4, 0:XC])
            nc.sync.dma_start(wq_sb[:, 4:8, :], wq_r[:, 4:8, :])
            nc.sync.dma_start(x_sb[:, 4:8, 0:XC], xt_r[:, 4:8, 0:XC])
            nc.sync.dma_start(p2_sb, p2t[:])
            nc.sync.dma_start(cos_sb, cosd[:])
            nc.sync.dma_start(sin_sb, sind[:])
            nc.sync.dma_start(wk_sb[:, 0:4, :], wk_r[:, 0:4, :])
            nc.sync.dma_start(wk_sb[:, 4:8, :], wk_r[:, 4:8, :])
            nc.sync.dma_start(x_sb[:, :, XC:2 * XC], xt_r[:, :, XC:2 * XC])
            # wv is needed by the first V unit at the start of chunk 1
            nc.sync.dma_start(wv_sb[:, 0:4, :], wv_r[:, 0:4, :])
            nc.sync.dma_start(wv_sb[:, 4:8, :], wv_r[:, 4:8, :])
            for c in range(2, NXC):
                cs = slice(c * XC, (c + 1) * XC)
                nc.sync.dma_start(x_sb[:, :, cs], xt_r[:, :, cs])

            def rope_stage(st):
                pre, dst, cs = st
                ps_a = psA.tile([128, XC], F32, tag="alt", name="ps_a")
                nc.tensor.matmul(ps_a, p2_sb, pre, start=True, stop=True)
                t1 = sA.tile([128, XC], F32, tag="t1", name="t1")
                nc.vector.tensor_mul(t1, sin_sb[:, cs], ps_a)
                t2 = sA.tile([128, XC], F32, tag="t2", name="t2")
                nc.gpsimd.tensor_mul(t2, cos_sb[:, cs], pre)
                nc.vector.tensor_add(dst[:, cs], t1, t2)

            for c in range(NXC):
                cs = slice(c * XC, (c + 1) * XC)
                pend = None
                if c > 0:
                    # pair-0 V for the previous chunk (deferred one chunk
                    # so the wv DMA has time to land)
                    v_unit(0, 2 * (c - 1))
                    v_unit(0, 2 * (c - 1) + 1)
                for (w_sb, dsts) in ((wq_sb, qt_t), (wk_sb, kt_t)):
                    for p in range(PAIRS):
                        ps = psA.tile([128, XC], F32, tag="proj", name="ps")
                        for dmt in range(DMT):
                            nc.tensor.matmul(
                                ps, w_sb[:, dmt, p * 128:(p + 1) * 128],
                                x_sb[:, dmt, cs],
                                start=(dmt == 0), stop=(dmt == DMT - 1))
                        pre = sA.tile([128, XC], BF16, tag="pre", name="pre")
                        nc.scalar.copy(pre, ps)
                        if pend is not None:
                            rope_stage(pend)
                        pend = (pre, dsts[p], cs)

                rope_stage(pend)
            v_unit(0, 14)
            v_unit(0, 15)

        # ---------------- Phase B: attention ----------------
        with tc.tile_pool(name="ep", bufs=7) as ep, \
             tc.tile_pool(name="h1p", bufs=2) as h1p, \
             tc.tile_pool(name="sB", bufs=3) as sB, \
             tc.tile_pool(name="shp", bufs=2) as shp, \
             tc.tile_pool(name="rbp", bufs=3) as rbp, \
             tc.tile_pool(name="psS", bufs=2, space="PSUM") as psS, \
             tc.tile_pool(name="psH", bufs=1, space="PSUM") as psH:

            # prefetch the out-projection weights while the PE chews on
            # attention — the DMA engines are nearly idle in phase B
            nc.sync.dma_start(
                wo_sb, wo.rearrange("(a p) m -> p a m", p=128))
            nc.sync.dma_start(wo2_sb, wo[448:512, :])

            stC = {"ysb": None}

            def c_unit(tt, ec):
                ts_ = slice(tt * 128, (tt + 1) * 128)
                es = slice(ec * 512, (ec + 1) * 512)
                if ec == 0:
                    stC["ysb"] = yst.tile([128, 2, 512], F32, tag="y",
                                          name="y_sb")
                y_sb = stC["ysb"]
                ps_y = psX.tile([128, 512], F32, tag="x", name="ps_y")
                h1l = stC.get("h1_last")
                if tt >= 12 and h1l is not None:
                    for p in range(PAIRS - 1):
                        nc.tensor.matmul(ps_y, h_t[p][:, ts_],
                                         wo_sb[:, p, es],
                                         start=(p == 0), stop=False)
                    nc.tensor.matmul(ps_y, h_t[3][0:64, ts_],
                                     wo_sb[0:64, 3, es],
                                     start=False, stop=False)
                    hs = slice(tt * 128 - 1536, tt * 128 - 1536 + 128)
                    nc.tensor.matmul(ps_y, h1l[0:64, hs], wo2_sb[:, es],
                                     start=False, stop=True)
                else:
                    for p in range(PAIRS):
                        nc.tensor.matmul(ps_y, h_t[p][:, ts_],
                                         wo_sb[:, p, es],
                                         start=(p == 0),
                                         stop=(p == PAIRS - 1))
                # keep the out-proj evacuations off ACT: it is saturated
                # with exps in these windows, and evac jitter there stalls
                # the score-PSUM ring
                if ec == 0:
                    nc.vector.tensor_copy(y_sb[:, 0, :], ps_y)
                    if tt >= 12:
                        # per-half stores at the very end shorten the drain
                        nc.sync.dma_start(y[ts_, 0:512], y_sb[:, 0, :])
                else:
                    nc.vector.tensor_copy(y_sb[:, 1, :], ps_y)
                    if tt >= 12:
                        nc.sync.dma_start(y[ts_, 512:1024], y_sb[:, 1, :])
                    else:
                        # one full-row DMA per t-tile
                        nc.sync.dma_start(y[ts_, :], y_sb)

            def emit_pv(st):
                p, ps_h, e_sb, c0, k, last = st
                nc.tensor.matmul(ps_h[0:65, 0, c0:],
                                 v_sb[:, k, 130 * p:130 * p + 65],
                                 e_sb[:, 0, c0:],
                                 start=(k == 0), stop=last)
                nc.tensor.matmul(ps_h[0:65, 1, c0:],
                                 v_sb[:, k, 130 * p + 65:130 * p + 130],
                                 e_sb[:, 1, c0:],
                                 start=(k == 0), stop=last)

            def emit_norm_head(p, ps_h, qc, h1, last=False):
                # evacuate H + sums to SBUF in one copy: ps_h (the single
                # PSUM PV buffer) frees after just this op, so the next
                # q-chunk's first PV is not gated on the whole chain.  For
                # the very last q-chunk there is no next PV — skip the
                # staging copy and read PSUM directly (shorter chain).
                if last:
                    s_h = ps_h
                else:
                    s_h = shp.tile([65, 2, 512], F32, tag="sh", name="s_h")
                    nc.vector.tensor_copy(s_h, ps_h[0:65, :, :])
                # the custom-DVE reciprocal misreads rows at non-zero base
                # partitions on hardware — stage the sums row at partition 0
                s1 = sB.tile([1, 2, 512], F32, tag="s1", name="s1")
                nc.vector.tensor_copy(s1, s_h[64:65, :, :])
                r1 = sB.tile([1, 2, 512], F32, tag="r1", name="r1")
                nc.vector.reciprocal_approx_fast(out=r1, in_=s1)
                r64 = rbp.tile([64, 2, 512], F32, tag="r64", name="r64")
                nc.gpsimd.partition_broadcast(r64, r1)
                return (p, qc, h1, s_h, r64)

            def emit_norm_tail(pend):
                # deferred: the DVE queue only sees these multiplies once
                # the Pool broadcast has surely landed (strict FIFOs — a
                # waiting instruction blocks everything behind it)
                p, qc, h1, s_h, r64 = pend
                qs = slice(qc * QC, (qc + 1) * QC)
                nc.vector.tensor_mul(h_t[p][0:64, qs], s_h[0:64, 0, :],
                                     r64[0:64, 0, :])
                nc.vector.tensor_mul(h1[0:64, :], s_h[0:64, 1, :],
                                     r64[0:64, 1, :])
                # odd head rows into partitions 64:128 of the pair tile
                nc.sync.dma_start(h_t[p][64:128, qs], h1[0:64, :])

            pend_pv = []
            pend_norm = None
            feed = []
            for p in range(PAIRS):
                if p < PAIRS - 1:
                    feed = [lambda pp=p + 1, kti=kti: v_unit(pp, kti)
                            for kti in range(NKT)]
                qt, kt = qt_t[p], kt_t[p]
                for qc in range(NQC):
                    nk = 4 * (qc + 1)
                    ps_h = psH.tile([128, 2, 512], F32, tag="pv", name="ps_h")
                    h1 = h1p.tile([64, QC], BF16, tag="h1", name="h1")
                    for k in range(nk):
                        m = k - 4 * qc
                        c0 = 128 * m if m >= 0 else 0
                        qs = slice(qc * QC + c0, (qc + 1) * QC)
                        ks = slice(k * KT, (k + 1) * KT)
                        ps_s = psS.tile([128, 2, 512], F32, tag="s",
                                        name="ps_s")
                        diag = m >= 0
                        nc.tensor.matmul(ps_s[:, 0, c0:], kt[0:64, ks],
                                         qt[0:64, qs], start=True,
                                         stop=True)
                        nc.tensor.matmul(ps_s[:, 1, c0:], kt[64:128, ks],
                                         qt[64:128, qs], start=True,
                                         stop=True)
                        e_sb = ep.tile([128, 2, 512], BF16, tag="e",
                                       name="e_sb")
                        nc.scalar.activation(e_sb[:, :, c0:], ps_s[:, :, c0:],
                                             AF.Exp, scale=0.125)
                        if diag:
                            # zero the strictly-upper-tri (k>q) entries of
                            # the diagonal block after exp
                            nc.gpsimd.affine_select(
                                e_sb[:, :, c0:c0 + 128],
                                e_sb[:, :, c0:c0 + 128],
                                pattern=[[0, 2], [1, 128]],
                                compare_op=ALU.is_ge,
                                fill=0.0, base=0, channel_multiplier=-1)
                        if len(pend_pv) >= 5:
                            emit_pv(pend_pv.pop(0))
                        pend_pv.append((p, ps_h, e_sb, c0, k, k == nk - 1))
                        # fire the deferred norm tail only once the Pool
                        # broadcast had time to land
                        if pend_norm is not None and k >= 3:
                            emit_norm_tail(pend_norm)
                            pend_norm = None
                        # PE filler: V projections of the next pair (or,
                        # for the last pair, out-projection tiles; k>=3 so
                        # they follow this q-chunk's norm tail)
                        if p < PAIRS - 1:
                            if feed and k % 5 in (1, 3):
                                feed.pop(0)()
                        elif k >= 5:
                            if feed:
                                feed.pop(0)()
                            if feed and k % 2 == 0:
                                feed.pop(0)()
                    # cover the last exp's latency, then drain the PVs and
                    # normalize this q-chunk
                    if feed and p < PAIRS - 1:
                        feed.pop(0)()
                    while pend_pv:
                        emit_pv(pend_pv.pop(0))
                    if pend_norm is not None:
                        emit_norm_tail(pend_norm)
                    if p == PAIRS - 1 and qc == NQC - 1:
                        stC["h1_last"] = h1
                    pend_norm = emit_norm_head(
                        p, ps_h, qc, h1,
                        last=(p == PAIRS - 1 and qc == NQC - 1))
                    if p == PAIRS - 1:
                        for tt in range(4 * qc, 4 * qc + 4):
                            feed.append(lambda tt=tt: c_unit(tt, 0))
                            feed.append(lambda tt=tt: c_unit(tt, 1))
                if p < PAIRS - 1:
                    # the next pair's attention needs its V complete
                    while feed:
                        feed.pop(0)()
            emit_norm_tail(pend_norm)
            # tail: remaining out-projection tiles
            while feed:
                feed.pop(0)()

        qkv_stack.close()

    nc.compile()
    return nc


def _host_prep(in_features, token_positions, Wq, Wk, Wv, Wo):
    """Shard + pre-transpose + bf16-cast inputs for the 8 cores."""
    x = np.asarray(in_features, dtype=np.float32)
    pos = np.asarray(token_positions)
    Wq = np.asarray(Wq, dtype=np.float32)
    Wk = np.asarray(Wk, dtype=np.float32)
    Wv = np.asarray(Wv, dtype=np.float32)
    Wo = np.asarray(Wo, dtype=np.float32)

    # rotate-half permutation of each head's 64 dims: evens then odds
    perm = np.concatenate([np.arange(0, 64, 2), np.arange(1, 64, 2)])
    full_perm = (np.arange(NUM_HEADS)[:, None] * 64 + perm[None, :]).reshape(-1)
    Wq_p = Wq[full_perm, :]   # permute output rows (head dims)
    Wk_p = Wk[full_perm, :]

    # P2: alt = P2 @ pre (per 64-block: alt[i] = -pre[32+i], alt[32+i]=pre[i])
    p2 = np.zeros((128, 128), np.float32)
    for blk in (0, 64):
        for i in range(32):
            p2[blk + i, blk + 32 + i] = -1.0
            p2[blk + 32 + i, blk + i] = 1.0
    p2t = np.ascontiguousarray(p2.T).astype(ml_dtypes.bfloat16)

    inv_freq = 1.0 / (THETA ** (np.arange(32, dtype=np.float64) * 2.0 / 64))

    bf = ml_dtypes.bfloat16
    in_maps = []
    for core in range(N_CORES):
        b = core // 2
        g = core % 2
        hs = slice(g * 512, (g + 1) * 512)   # head-dim slice of d_model

        ang = pos[b].astype(np.float64)[None, :] * inv_freq[:, None]  # [32,T]
        cos64 = np.cos(ang).astype(np.float32)
        sin64 = np.sin(ang).astype(np.float32)
        cos128 = np.tile(np.concatenate([cos64, cos64], 0), (2, 1))   # [128,T]
        sin128 = np.tile(np.concatenate([sin64, sin64], 0), (2, 1))

        in_maps.append({
            "xt": np.ascontiguousarray(x[b].T).astype(bf),
            "wq": np.ascontiguousarray(Wq_p[hs, :].T).astype(bf),
            "wk": np.ascontiguousarray(Wk_p[hs, :].T).astype(bf),
            "wv": np.ascontiguousarray(Wv[hs, :].T).astype(bf),
            "wo": np.ascontiguousarray(Wo[:, hs].T).astype(bf),
            "cos": np.ascontiguousarray(cos128).astype(bf),
            "sin": np.ascontiguousarray(sin128).astype(bf),
            "p2t": p2t,
        })
    return in_maps


def kernel(**inputs):
    if "nc" not in _CACHE:
        _CACHE["nc"] = _build_nc()
    nc = _CACHE["nc"]
    in_maps = _host_prep(**inputs)
    res = run_bass_kernel_spmd(nc, in_maps, core_ids=list(range(N_CORES)))
    out = np.zeros((B, T, D_MODEL), np.float32)
    for core in range(N_CORES):
        out[core // 2] += res.results[core]["y"]
    return out


# revision 24
# speedup vs baseline: 1.0630x; 1.0043x over previous
"""Causal multi-head self-attention (RoPE) Trainium2 Bass kernel.

Problem: B=4, T=2048, d_model=1024, 16 heads, d_k=64, causal, RoPE,
fp32 I/O.  Sharding: batch (4-way) x head-group (2-way) over 8 cores.
Core c handles batch c//2 and heads [8*(c%2) .. 8*(c%2)+8).

Everything on device runs in the "transposed" domain to avoid on-chip
transposes entirely:
  QT/KT:  [head_dim, T]   (head dim on partitions), bf16
  V:      [T, head_dim]   (k positions on partitions), bf16 + ones col
  scores: S^T [k, q] = KT_tile^T @ QT  (per head), fp32 PSUM
  E = exp(S^T/8) in bf16; causal masking of the diagonal blocks via
  gpsimd affine_select (zero out k>q entries after exp)
  PV: H^T[d, q] = [V|ones]^T @ E  -> heads rows 0:64 + sums row 64
  normalize via fast reciprocal of the sums row + gpsimd
  partition_broadcast + DVE multiplies
  out-proj: y[t, e] = H^T_tile^T @ WoT, accumulated over head pairs

All matmuls run in bf16 (1 cycle/row at any moving size); accumulation
is fp32 in PSUM.  RoPE is applied as rot = cos (.) pre + sin (.)
(P2 @ pre), where the head dim has been host-permuted to rotate-half
layout and P2 is the fixed swap-negate permutation folded into a
128x128 matmul.

The k-loop and projection loops are software-pipelined by one stage so
the PE never stalls on ACT (exp / psum evacuation) latency.
"""

import numpy as np
from contextlib import ExitStack

import concourse.bass as bass
from concourse import bacc
import concourse.tile as tile
import concourse.mybir as mybir
import ml_dtypes
from concourse.bass_utils import run_bass_kernel_spmd

F32 = mybir.dt.float32
BF16 = mybir.dt.bfloat16
AF = mybir.ActivationFunctionType
ALU = mybir.AluOpType

D_MODEL = 1024
NUM_HEADS = 16
THETA = 10000.0
B, T = 4, 2048
N_CORES = 8
PAIRS = 4             # head pairs per core (8 heads)
QC = 512              # q-chunk width
NQC = T // QC
KT = 128              # k-tile height
NKT = T // KT
XC = 256              # xt streaming chunk width (phase A)
NXC = T // XC
DMT = D_MODEL // 128  # 8 d_model k-tiles

_CACHE = {}


def _build_nc():
    nc = bacc.Bacc(None, target_bir_lowering=False)

    xt = nc.dram_tensor("xt", [D_MODEL, T], BF16, kind="ExternalInput")
    wq = nc.dram_tensor("wq", [D_MODEL, 512], BF16, kind="ExternalInput")
    wk = nc.dram_tensor("wk", [D_MODEL, 512], BF16, kind="ExternalInput")
    wv = nc.dram_tensor("wv", [D_MODEL, 512], BF16, kind="ExternalInput")
    wo = nc.dram_tensor("wo", [512, D_MODEL], BF16, kind="ExternalInput")
    cosd = nc.dram_tensor("cos", [128, T], BF16, kind="ExternalInput")
    sind = nc.dram_tensor("sin", [128, T], BF16, kind="ExternalInput")
    p2t = nc.dram_tensor("p2t", [128, 128], BF16, kind="ExternalInput")
    y = nc.dram_tensor("y", [T, D_MODEL], F32, kind="ExternalOutput")

    with tile.TileContext(nc) as tc, ExitStack() as ctx:
        constp = ctx.enter_context(tc.tile_pool(name="const", bufs=1))
        qkv_stack = ExitStack()
        qkp = qkv_stack.enter_context(tc.tile_pool(name="qk", bufs=1))
        vp = qkv_stack.enter_context(tc.tile_pool(name="v", bufs=1))

        cos_sb = constp.tile([128, T], BF16)
        sin_sb = constp.tile([128, T], BF16)
        p2_sb = constp.tile([128, 128], BF16)
        # wo lives in a long-lived pool so it can be prefetched in phase B
        wo_sb = constp.tile([128, PAIRS, D_MODEL], BF16)
        # pair-3 odd-head rows of wo staged at partitions 0:64 so the final
        # out-proj tiles can read the h1 tile directly (skipping the h1->h_t
        # DMA on the tail critical path)
        wo2_sb = constp.tile([64, D_MODEL], BF16)

        qt_t = [qkp.tile([128, T], BF16, name=f"qt{p}", tag=f"qt{p}")
                for p in range(PAIRS)]
        kt_t = [qkp.tile([128, T], BF16, name=f"kt{p}", tag=f"kt{p}")
                for p in range(PAIRS)]
        # V with interleaved ones cols: per k-tile, per pair:
        # [V_h0(64) | 1 | V_h1(64) | 1] = 130 cols
        v_sb = vp.tile([128, NKT, PAIRS * 130], BF16)
        v5 = v_sb.rearrange("p t (pr x) -> p t pr x", pr=PAIRS)
        v6 = v5.rearrange("p t pr (hl c) -> p t pr hl c", hl=2)
        nc.vector.memset(v6[:, :, :, :, 64:65], 1.0)

        # x and wv stay resident through phase B: the V projections of
        # pairs 1-3 are interleaved into the attention windows (which are
        # Activation-bound) as pure-PE filler work
        xp = qkv_stack.enter_context(tc.tile_pool(name="xp", bufs=1))
        wvp = qkv_stack.enter_context(tc.tile_pool(name="wvp", bufs=1))
        x_sb = xp.tile([128, DMT, T], BF16)
        wv_sb = wvp.tile([128, DMT, 512], BF16)
        psX = qkv_stack.enter_context(tc.tile_pool(name="psX", bufs=2,
                                                   space="PSUM"))
        yst = qkv_stack.enter_context(tc.tile_pool(name="yst", bufs=3))
        hp = qkv_stack.enter_context(
            tc.tile_pool(name="hp", bufs=1, side="right"))
        h_t = [hp.tile([128, T], BF16, name=f"h{p}", tag=f"h{p}")
               for p in range(PAIRS)]

        def v_unit(pp, kti):
            ps_x = psX.tile([128, 512], F32, tag="x", name="ps_v")
            ps_v = ps_x[:, 0:128]
            for dmt in range(DMT):
                nc.tensor.matmul(
                    ps_v, x_sb[:, dmt, kti * 128:(kti + 1) * 128],
                    wv_sb[:, dmt, pp * 128:(pp + 1) * 128],
                    start=(dmt == 0), stop=(dmt == DMT - 1))
            src = ps_v.rearrange("p (hl c) -> p hl c", hl=2)
            nc.vector.tensor_copy(v6[:, kti, pp, :, 0:64], src)

        # ---------------- Phase A: projections + RoPE ----------------
        with tc.tile_pool(name="wqkv", bufs=1) as wp, \
             tc.tile_pool(name="sA", bufs=5) as sA, \
             tc.tile_pool(name="psA", bufs=3, space="PSUM") as psA:
            wq_sb = wp.tile([128, DMT, 512], BF16)
            wk_sb = wp.tile([128, DMT, 512], BF16)
            xt_r = xt.rearrange("(a p) t -> p a t", p=128)
            wq_r = wq.rearrange("(a p) m -> p a m", p=128)
            wk_r = wk.rearrange("(a p) m -> p a m", p=128)
            wv_r = wv.rearrange("(a p) m -> p a m", p=128)
            # priority order: interleave wq + first x chunk per-dmt so the
            # first matmul group can start as soon as its slices land;
            # everything else trickles in behind them.
            # consolidated prefix: every DMA pays ~625ns of HWDGE, so
            # fewer, larger transfers get wk/cos/sin on chip sooner
            nc.sync.dma_start(wq_sb[:, 0:4, :], wq_r[:, 0:4, :])
            nc.sync.dma_start(x_sb[:, 0:4, 0:XC], xt_r[:, 0:4, 0:XC])
            nc.sync.dma_start(wq_sb[:, 4:8, :], wq_r[:, 4:8, :])
            nc.sync.dma_start(x_sb[:, 4:8, 0:XC], xt_r[:, 4:8, 0:XC])
            nc.sync.dma_start(p2_sb, p2t[:])
            nc.sync.dma_start(cos_sb, cosd[:])
            nc.sync.dma_start(sin_sb, sind[:])
            nc.sync.dma_start(wk_sb[:, 0:4, :], wk_r[:, 0:4, :])
            nc.sync.dma_start(wk_sb[:, 4:8, :], wk_r[:, 4:8, :])
            nc.sync.dma_start(x_sb[:, :, XC:2 * XC], xt_r[:, :, XC:2 * XC])
            # wv is needed by the first V unit at the start of chunk 1
            nc.sync.dma_start(wv_sb[:, 0:4, :], wv_r[:, 0:4, :])
            nc.sync.dma_start(wv_sb[:, 4:8, :], wv_r[:, 4:8, :])
            for c in range(2, NXC):
                cs = slice(c * XC, (c + 1) * XC)
                nc.sync.dma_start(x_sb[:, :, cs], xt_r[:, :, cs])

            def rope_stage(st):
                pre, dst, cs = st
                ps_a = psA.tile([128, XC], F32, tag="alt", name="ps_a")
                nc.tensor.matmul(ps_a, p2_sb, pre, start=True, stop=True)
                t1 = sA.tile([128, XC], F32, tag="t1", name="t1")
                nc.vector.tensor_mul(t1, sin_sb[:, cs], ps_a)
                t2 = sA.tile([128, XC], F32, tag="t2", name="t2")
                nc.gpsimd.tensor_mul(t2, cos_sb[:, cs], pre)
                nc.vector.tensor_add(dst[:, cs], t1, t2)

            for c in range(NXC):
                cs = slice(c * XC, (c + 1) * XC)
                pend = None
                if c > 0:
                    # pair-0 V for the previous chunk (deferred one chunk
                    # so the wv DMA has time to land)
                    v_unit(0, 2 * (c - 1))
                    v_unit(0, 2 * (c - 1) + 1)
                for (w_sb, dsts) in ((wq_sb, qt_t), (wk_sb, kt_t)):
                    for p in range(PAIRS):
                        ps = psA.tile([128, XC], F32, tag="proj", name="ps")
                        for dmt in range(DMT):
                            nc.tensor.matmul(
                                ps, w_sb[:, dmt, p * 128:(p + 1) * 128],
                                x_sb[:, dmt, cs],
                                start=(dmt == 0), stop=(dmt == DMT - 1))
                        pre = sA.tile([128, XC], BF16, tag="pre", name="pre")
                        nc.scalar.copy(pre, ps)
                        if pend is not None:
                            rope_stage(pend)
                        pend = (pre, dsts[p], cs)

                rope_stage(pend)
            v_unit(0, 14)
            v_unit(0, 15)

        # ---------------- Phase B: attention ----------------
        with tc.tile_pool(name="ep", bufs=7) as ep, \
             tc.tile_pool(name="h1p", bufs=2) as h1p, \
             tc.tile_pool(name="sB", bufs=3) as sB, \
             tc.tile_pool(name="shp", bufs=2) as shp, \
             tc.tile_pool(name="rbp", bufs=3) as rbp, \
             tc.tile_pool(name="psS", bufs=2, space="PSUM") as psS, \
             tc.tile_pool(name="psH", bufs=1, space="PSUM") as psH:

            # prefetch the out-projection weights while the PE chews on
            # attention — the DMA engines are nearly idle in phase B
            nc.sync.dma_start(
                wo_sb, wo.rearrange("(a p) m -> p a m", p=128))
            nc.sync.dma_start(wo2_sb, wo[448:512, :])

            stC = {"ysb": None}

            def c_unit(tt, ec):
                ts_ = slice(tt * 128, (tt + 1) * 128)
                es = slice(ec * 512, (ec + 1) * 512)
                if ec == 0:
                    stC["ysb"] = yst.tile([128, 2, 512], F32, tag="y",
                                          name="y_sb")
                y_sb = stC["ysb"]
                ps_y = psX.tile([128, 512], F32, tag="x", name="ps_y")
                h1l = stC.get("h1_last")
                if tt >= 12 and h1l is not None:
                    for p in range(PAIRS - 1):
                        nc.tensor.matmul(ps_y, h_t[p][:, ts_],
                                         wo_sb[:, p, es],
                                         start=(p == 0), stop=False)
                    nc.tensor.matmul(ps_y, h_t[3][0:64, ts_],
                                     wo_sb[0:64, 3, es],
                                     start=False, stop=False)
                    hs = slice(tt * 128 - 1536, tt * 128 - 1536 + 128)
                    nc.tensor.matmul(ps_y, h1l[0:64, hs], wo2_sb[:, es],
                                     start=False, stop=True)
                else:
                    for p in range(PAIRS):
                        nc.tensor.matmul(ps_y, h_t[p][:, ts_],
                                         wo_sb[:, p, es],
                                         start=(p == 0),
                                         stop=(p == PAIRS - 1))
                # keep the out-proj evacuations off ACT: it is saturated
                # with exps in these windows, and evac jitter there stalls
                # the score-PSUM ring
                if ec == 0:
                    nc.vector.tensor_copy(y_sb[:, 0, :], ps_y)
                    if tt >= 12:
                        # per-half stores at the very end shorten the drain
                        nc.sync.dma_start(y[ts_, 0:512], y_sb[:, 0, :])
                else:
                    nc.vector.tensor_copy(y_sb[:, 1, :], ps_y)
                    if tt >= 12:
                        nc.sync.dma_start(y[ts_, 512:1024], y_sb[:, 1, :])
                    else:
                        # one full-row DMA per t-tile
                        nc.sync.dma_start(y[ts_, :], y_sb)

            def emit_pv(st):
                p, ps_h, e_sb, c0, k, first, last = st
                nc.tensor.matmul(ps_h[0:65, 0, c0:],
                                 v_sb[:, k, 130 * p:130 * p + 65],
                                 e_sb[:, 0, c0:],
                                 start=first, stop=last)
                nc.tensor.matmul(ps_h[0:65, 1, c0:],
                                 v_sb[:, k, 130 * p + 65:130 * p + 130],
                                 e_sb[:, 1, c0:],
                                 start=first, stop=last)

            def emit_norm_head(p, ps_h, qc, h1, last=False):
                # evacuate H + sums to SBUF in one copy: ps_h (the single
                # PSUM PV buffer) frees after just this op, so the next
                # q-chunk's first PV is not gated on the whole chain.  For
                # the very last q-chunk there is no next PV — skip the
                # staging copy and read PSUM directly (shorter chain).
                if last:
                    s_h = ps_h
                else:
                    s_h = shp.tile([65, 2, 512], F32, tag="sh", name="s_h")
                    nc.vector.tensor_copy(s_h, ps_h[0:65, :, :])
                # the custom-DVE reciprocal misreads rows at non-zero base
                # partitions on hardware — stage the sums row at partition 0
                s1 = sB.tile([1, 2, 512], F32, tag="s1", name="s1")
                nc.vector.tensor_copy(s1, s_h[64:65, :, :])
                r1 = sB.tile([1, 2, 512], F32, tag="r1", name="r1")
                nc.vector.reciprocal_approx_fast(out=r1, in_=s1)
                r64 = rbp.tile([64, 2, 512], F32, tag="r64", name="r64")
                nc.gpsimd.partition_broadcast(r64, r1)
                return (p, qc, h1, s_h, r64)

            def emit_norm_tail(pend):
                # deferred: the DVE queue only sees these multiplies once
                # the Pool broadcast has surely landed (strict FIFOs — a
                # waiting instruction blocks everything behind it)
                p, qc, h1, s_h, r64 = pend
                qs = slice(qc * QC, (qc + 1) * QC)
                nc.vector.tensor_mul(h_t[p][0:64, qs], s_h[0:64, 0, :],
                                     r64[0:64, 0, :])
                nc.vector.tensor_mul(h1[0:64, :], s_h[0:64, 1, :],
                                     r64[0:64, 1, :])
                # odd head rows into partitions 64:128 of the pair tile
                nc.sync.dma_start(h_t[p][64:128, qs], h1[0:64, :])

            pend_pv = []
            pend_norm = None
            feed = []
            for p in range(PAIRS):
                if p < PAIRS - 1:
                    feed = [lambda pp=p + 1, kti=kti: v_unit(pp, kti)
                            for kti in range(NKT)]
                qt, kt = qt_t[p], kt_t[p]
                for qc in range(NQC):
                    nk = 4 * (qc + 1)
                    ps_h = psH.tile([128, 2, 512], F32, tag="pv", name="ps_h")
                    h1 = h1p.tile([64, QC], BF16, tag="h1", name="h1")
                    # process the diagonal k-tiles FIRST: their
                    # affine_selects (Pool) then hide behind the rest of
                    # the window instead of gating the end-of-chunk drain
                    k_order = list(range(4 * qc, nk)) + list(range(4 * qc))
                    for ki, k in enumerate(k_order):
                        m = k - 4 * qc
                        c0 = 128 * m if m >= 0 else 0
                        qs = slice(qc * QC + c0, (qc + 1) * QC)
                        ks = slice(k * KT, (k + 1) * KT)
                        ps_s = psS.tile([128, 2, 512], F32, tag="s",
                                        name="ps_s")
                        diag = m >= 0
                        nc.tensor.matmul(ps_s[:, 0, c0:], kt[0:64, ks],
                                         qt[0:64, qs], start=True,
                                         stop=True)
                        nc.tensor.matmul(ps_s[:, 1, c0:], kt[64:128, ks],
                                         qt[64:128, qs], start=True,
                                         stop=True)
                        e_sb = ep.tile([128, 2, 512], BF16, tag="e",
                                       name="e_sb")
                        nc.scalar.activation(e_sb[:, :, c0:], ps_s[:, :, c0:],
                                             AF.Exp, scale=0.125)
                        if diag:
                            # zero the strictly-upper-tri (k>q) entries of
                            # the diagonal block after exp
                            nc.gpsimd.affine_select(
                                e_sb[:, :, c0:c0 + 128],
                                e_sb[:, :, c0:c0 + 128],
                                pattern=[[0, 2], [1, 128]],
                                compare_op=ALU.is_ge,
                                fill=0.0, base=0, channel_multiplier=-1)
                        if len(pend_pv) >= 5:
                            emit_pv(pend_pv.pop(0))
                        pend_pv.append((p, ps_h, e_sb, c0, k,
                                        ki == 0, ki == nk - 1))
                        # fire the deferred norm tail only once the Pool
                        # broadcast had time to land
                        if pend_norm is not None and ki >= 3:
                            emit_norm_tail(pend_norm)
                            pend_norm = None
                        # PE filler: V projections of the next pair (or,
                        # for the last pair, out-projection tiles; ki>=5 so
                        # they follow this q-chunk's norm tail)
                        if p < PAIRS - 1:
                            if feed and ki % 5 in (1, 3):
                                feed.pop(0)()
                        elif ki >= 5:
                            if feed:
                                feed.pop(0)()
                            if feed and ki % 2 == 0:
                                feed.pop(0)()
                    # cover the last exp's latency, then drain the PVs and
                    # normalize this q-chunk
                    if feed and p < PAIRS - 1:
                        feed.pop(0)()
                    while pend_pv:
                        emit_pv(pend_pv.pop(0))
                    if pend_norm is not None:
                        emit_norm_tail(pend_norm)
                    if p == PAIRS - 1 and qc == NQC - 1:
                        stC["h1_last"] = h1
                    pend_norm = emit_norm_head(
                        p, ps_h, qc, h1,
                        last=(p == PAIRS - 1 and qc == NQC - 1))
                    if p == PAIRS - 1:
                        for tt in range(4 * qc, 4 * qc + 4):
                            feed.append(lambda tt=tt: c_unit(tt, 0))
                            feed.append(lambda tt=tt: c_unit(tt, 1))
                if p < PAIRS - 1:
                    # the next pair's attention needs its V complete
                    while feed:
                        feed.pop(0)()
            emit_norm_tail(pend_norm)
            # tail: remaining out-projection tiles
            while feed:
                feed.pop(0)()

        qkv_stack.close()

    nc.compile()
    return nc


def _host_prep(in_features, token_positions, Wq, Wk, Wv, Wo):
    """Shard + pre-transpose + bf16-cast inputs for the 8 cores."""
    x = np.asarray(in_features, dtype=np.float32)
    pos = np.asarray(token_positions)
    Wq = np.asarray(Wq, dtype=np.float32)
    Wk = np.asarray(Wk, dtype=np.float32)
    Wv = np.asarray(Wv, dtype=np.float32)
    Wo = np.asarray(Wo, dtype=np.float32)

    # rotate-half permutation of each head's 64 dims: evens then odds
    perm = np.concatenate([np.arange(0, 64, 2), np.arange(1, 64, 2)])
    full_perm = (np.arange(NUM_HEADS)[:, None] * 64 + perm[None, :]).reshape(-1)
    Wq_p = Wq[full_perm, :]   # permute output rows (head dims)
    Wk_p = Wk[full_perm, :]

    # P2: alt = P2 @ pre (per 64-block: alt[i] = -pre[32+i], alt[32+i]=pre[i])
    p2 = np.zeros((128, 128), np.float32)
    for blk in (0, 64):
        for i in range(32):
            p2[blk + i, blk + 32 + i] = -1.0
            p2[blk + 32 + i, blk + i] = 1.0
    p2t = np.ascontiguousarray(p2.T).astype(ml_dtypes.bfloat16)

    inv_freq = 1.0 / (THETA ** (np.arange(32, dtype=np.float64) * 2.0 / 64))

    bf = ml_dtypes.bfloat16
    in_maps = []
    for core in range(N_CORES):
        b = core // 2
        g = core % 2
        hs = slice(g * 512, (g + 1) * 512)   # head-dim slice of d_model

        ang = pos[b].astype(np.float64)[None, :] * inv_freq[:, None]  # [32,T]
        cos64 = np.cos(ang).astype(np.float32)
        sin64 = np.sin(ang).astype(np.float32)
        cos128 = np.tile(np.concatenate([cos64, cos64], 0), (2, 1))   # [128,T]
        sin128 = np.tile(np.concatenate([sin64, sin64], 0), (2, 1))

        in_maps.append({
            "xt": np.ascontiguousarray(x[b].T).astype(bf),
            "wq": np.ascontiguousarray(Wq_p[hs, :].T).astype(bf),
            "wk": np.ascontiguousarray(Wk_p[hs, :].T).astype(bf),
            "wv": np.ascontiguousarray(Wv[hs, :].T).astype(bf),
            "wo": np.ascontiguousarray(Wo[:, hs].T).astype(bf),
            "cos": np.ascontiguousarray(cos128).astype(bf),
            "sin": np.ascontiguousarray(sin128).astype(bf),
            "p2t": p2t,
        })
    return in_maps


def kernel(**inputs):
    if "nc" not in _CACHE:
        _CACHE["nc"] = _build_nc()
    nc = _CACHE["nc"]
    in_maps = _host_prep(**inputs)
    res = run_bass_kernel_spmd(nc, in_maps, core_ids=list(range(N_CORES)))
    out = np.zeros((B, T, D_MODEL), np.float32)
    for core in range(N_CORES):
        out[core // 2] += res.results[core]["y"]
    return out


# revision 25
# speedup vs baseline: 1.0641x; 1.0011x over previous
"""Causal multi-head self-attention (RoPE) Trainium2 Bass kernel.

Problem: B=4, T=2048, d_model=1024, 16 heads, d_k=64, causal, RoPE,
fp32 I/O.  Sharding: batch (4-way) x head-group (2-way) over 8 cores.
Core c handles batch c//2 and heads [8*(c%2) .. 8*(c%2)+8).

Everything on device runs in the "transposed" domain to avoid on-chip
transposes entirely:
  QT/KT:  [head_dim, T]   (head dim on partitions), bf16
  V:      [T, head_dim]   (k positions on partitions), bf16 + ones col
  scores: S^T [k, q] = KT_tile^T @ QT  (per head), fp32 PSUM
  E = exp(S^T/8) in bf16; causal masking of the diagonal blocks via
  gpsimd affine_select (zero out k>q entries after exp)
  PV: H^T[d, q] = [V|ones]^T @ E  -> heads rows 0:64 + sums row 64
  normalize via fast reciprocal of the sums row + gpsimd
  partition_broadcast + DVE multiplies
  out-proj: y[t, e] = H^T_tile^T @ WoT, accumulated over head pairs

All matmuls run in bf16 (1 cycle/row at any moving size); accumulation
is fp32 in PSUM.  RoPE is applied as rot = cos (.) pre + sin (.)
(P2 @ pre), where the head dim has been host-permuted to rotate-half
layout and P2 is the fixed swap-negate permutation folded into a
128x128 matmul.

The k-loop and projection loops are software-pipelined by one stage so
the PE never stalls on ACT (exp / psum evacuation) latency.
"""

import numpy as np
from contextlib import ExitStack

import concourse.bass as bass
from concourse import bacc
import concourse.tile as tile
import concourse.mybir as mybir
import ml_dtypes
from concourse.bass_utils import run_bass_kernel_spmd

F32 = mybir.dt.float32
BF16 = mybir.dt.bfloat16
AF = mybir.ActivationFunctionType
ALU = mybir.AluOpType

D_MODEL = 1024
NUM_HEADS = 16
THETA = 10000.0
B, T = 4, 2048
N_CORES = 8
PAIRS = 4             # head pairs per core (8 heads)
QC = 512              # q-chunk width
NQC = T // QC
KT = 128              # k-tile height
NKT = T // KT
XC = 256              # xt streaming chunk width (phase A)
NXC = T // XC
DMT = D_MODEL // 128  # 8 d_model k-tiles

_CACHE = {}


def _build_nc():
    nc = bacc.Bacc(None, target_bir_lowering=False)

    xt = nc.dram_tensor("xt", [D_MODEL, T], BF16, kind="ExternalInput")
    wq = nc.dram_tensor("wq", [D_MODEL, 512], BF16, kind="ExternalInput")
    wk = nc.dram_tensor("wk", [D_MODEL, 512], BF16, kind="ExternalInput")
    wv = nc.dram_tensor("wv", [D_MODEL, 512], BF16, kind="ExternalInput")
    wo = nc.dram_tensor("wo", [512, D_MODEL], BF16, kind="ExternalInput")
    cosd = nc.dram_tensor("cos", [128, T], BF16, kind="ExternalInput")
    sind = nc.dram_tensor("sin", [128, T], BF16, kind="ExternalInput")
    p2t = nc.dram_tensor("p2t", [128, 128], BF16, kind="ExternalInput")
    y = nc.dram_tensor("y", [T, D_MODEL], F32, kind="ExternalOutput")

    with tile.TileContext(nc) as tc, ExitStack() as ctx:
        constp = ctx.enter_context(tc.tile_pool(name="const", bufs=1))
        qkv_stack = ExitStack()
        qkp = qkv_stack.enter_context(tc.tile_pool(name="qk", bufs=1))
        vp = qkv_stack.enter_context(tc.tile_pool(name="v", bufs=1))

        cos_sb = constp.tile([128, T], BF16)
        sin_sb = constp.tile([128, T], BF16)
        p2_sb = constp.tile([128, 128], BF16)
        # wo lives in a long-lived pool so it can be prefetched in phase B
        wo_sb = constp.tile([128, PAIRS, D_MODEL], BF16)
        # pair-3 odd-head rows of wo staged at partitions 0:64 so the final
        # out-proj tiles can read the h1 tile directly (skipping the h1->h_t
        # DMA on the tail critical path)
        wo2_sb = constp.tile([64, D_MODEL], BF16)

        qt_t = [qkp.tile([128, T], BF16, name=f"qt{p}", tag=f"qt{p}")
                for p in range(PAIRS)]
        kt_t = [qkp.tile([128, T], BF16, name=f"kt{p}", tag=f"kt{p}")
                for p in range(PAIRS)]
        # V with interleaved ones cols: per k-tile, per pair:
        # [V_h0(64) | 1 | V_h1(64) | 1] = 130 cols
        v_sb = vp.tile([128, NKT, PAIRS * 130], BF16)
        v5 = v_sb.rearrange("p t (pr x) -> p t pr x", pr=PAIRS)
        v6 = v5.rearrange("p t pr (hl c) -> p t pr hl c", hl=2)
        nc.vector.memset(v6[:, :, :, :, 64:65], 1.0)

        # x and wv stay resident through phase B: the V projections of
        # pairs 1-3 are interleaved into the attention windows (which are
        # Activation-bound) as pure-PE filler work
        xp = qkv_stack.enter_context(tc.tile_pool(name="xp", bufs=1))
        wvp = qkv_stack.enter_context(tc.tile_pool(name="wvp", bufs=1))
        x_sb = xp.tile([128, DMT, T], BF16)
        wv_sb = wvp.tile([128, DMT, 512], BF16)
        psX = qkv_stack.enter_context(tc.tile_pool(name="psX", bufs=2,
                                                   space="PSUM"))
        yst = qkv_stack.enter_context(tc.tile_pool(name="yst", bufs=3))
        hp = qkv_stack.enter_context(
            tc.tile_pool(name="hp", bufs=1, side="right"))
        h_t = [hp.tile([128, T], BF16, name=f"h{p}", tag=f"h{p}")
               for p in range(PAIRS)]

        def v_unit(pp, kti):
            ps_x = psX.tile([128, 512], F32, tag="x", name="ps_v")
            ps_v = ps_x[:, 0:128]
            for dmt in range(DMT):
                nc.tensor.matmul(
                    ps_v, x_sb[:, dmt, kti * 128:(kti + 1) * 128],
                    wv_sb[:, dmt, pp * 128:(pp + 1) * 128],
                    start=(dmt == 0), stop=(dmt == DMT - 1))
            src = ps_v.rearrange("p (hl c) -> p hl c", hl=2)
            nc.vector.tensor_copy(v6[:, kti, pp, :, 0:64], src)

        # ---------------- Phase A: projections + RoPE ----------------
        with tc.tile_pool(name="wqkv", bufs=1) as wp, \
             tc.tile_pool(name="sA", bufs=5) as sA, \
             tc.tile_pool(name="psA", bufs=3, space="PSUM") as psA:
            wq_sb = wp.tile([128, DMT, 512], BF16)
            wk_sb = wp.tile([128, DMT, 512], BF16)
            xt_r = xt.rearrange("(a p) t -> p a t", p=128)
            wq_r = wq.rearrange("(a p) m -> p a m", p=128)
            wk_r = wk.rearrange("(a p) m -> p a m", p=128)
            wv_r = wv.rearrange("(a p) m -> p a m", p=128)
            # priority order: interleave wq + first x chunk per-dmt so the
            # first matmul group can start as soon as its slices land;
            # everything else trickles in behind them.
            # consolidated prefix: every DMA pays ~625ns of HWDGE, so
            # fewer, larger transfers get wk/cos/sin on chip sooner
            nc.sync.dma_start(wq_sb[:, 0:4, :], wq_r[:, 0:4, :])
            nc.sync.dma_start(x_sb[:, 0:4, 0:XC], xt_r[:, 0:4, 0:XC])
            nc.sync.dma_start(wq_sb[:, 4:8, :], wq_r[:, 4:8, :])
            nc.sync.dma_start(x_sb[:, 4:8, 0:XC], xt_r[:, 4:8, 0:XC])
            nc.sync.dma_start(p2_sb, p2t[:])
            nc.sync.dma_start(cos_sb, cosd[:])
            nc.sync.dma_start(sin_sb, sind[:])
            nc.sync.dma_start(wk_sb[:, 0:4, :], wk_r[:, 0:4, :])
            nc.sync.dma_start(wk_sb[:, 4:8, :], wk_r[:, 4:8, :])
            nc.sync.dma_start(x_sb[:, :, XC:2 * XC], xt_r[:, :, XC:2 * XC])
            # wv is needed by the first V unit at the start of chunk 1
            nc.sync.dma_start(wv_sb[:, 0:4, :], wv_r[:, 0:4, :])
            nc.sync.dma_start(wv_sb[:, 4:8, :], wv_r[:, 4:8, :])
            for c in range(2, NXC):
                cs = slice(c * XC, (c + 1) * XC)
                nc.sync.dma_start(x_sb[:, :, cs], xt_r[:, :, cs])

            def rope_stage(st):
                pre, dst, cs = st
                ps_a = psA.tile([128, XC], F32, tag="alt", name="ps_a")
                nc.tensor.matmul(ps_a, p2_sb, pre, start=True, stop=True)
                t1 = sA.tile([128, XC], F32, tag="t1", name="t1")
                nc.vector.tensor_mul(t1, sin_sb[:, cs], ps_a)
                t2 = sA.tile([128, XC], F32, tag="t2", name="t2")
                nc.gpsimd.tensor_mul(t2, cos_sb[:, cs], pre)
                nc.vector.tensor_add(dst[:, cs], t1, t2)

            for c in range(NXC):
                cs = slice(c * XC, (c + 1) * XC)
                pend = None
                if c > 0:
                    # pair-0 V for the previous chunk (deferred one chunk
                    # so the wv DMA has time to land)
                    v_unit(0, 2 * (c - 1))
                    v_unit(0, 2 * (c - 1) + 1)
                for (w_sb, dsts) in ((wq_sb, qt_t), (wk_sb, kt_t)):
                    for p in range(PAIRS):
                        ps = psA.tile([128, XC], F32, tag="proj", name="ps")
                        for dmt in range(DMT):
                            nc.tensor.matmul(
                                ps, w_sb[:, dmt, p * 128:(p + 1) * 128],
                                x_sb[:, dmt, cs],
                                start=(dmt == 0), stop=(dmt == DMT - 1))
                        pre = sA.tile([128, XC], BF16, tag="pre", name="pre")
                        nc.scalar.copy(pre, ps)
                        if pend is not None:
                            rope_stage(pend)
                        pend = (pre, dsts[p], cs)

                rope_stage(pend)
            v_unit(0, 14)
            v_unit(0, 15)

        # ---------------- Phase B: attention ----------------
        with tc.tile_pool(name="ep", bufs=7) as ep, \
             tc.tile_pool(name="h1p", bufs=2) as h1p, \
             tc.tile_pool(name="sB", bufs=3) as sB, \
             tc.tile_pool(name="shp", bufs=2) as shp, \
             tc.tile_pool(name="rbp", bufs=3) as rbp, \
             tc.tile_pool(name="psS", bufs=2, space="PSUM") as psS, \
             tc.tile_pool(name="psH", bufs=1, space="PSUM") as psH:

            # prefetch the out-projection weights while the PE chews on
            # attention — the DMA engines are nearly idle in phase B
            nc.sync.dma_start(
                wo_sb, wo.rearrange("(a p) m -> p a m", p=128))
            nc.sync.dma_start(wo2_sb, wo[448:512, :])

            stC = {"ysb": None}

            def c_unit(tt, ec):
                ts_ = slice(tt * 128, (tt + 1) * 128)
                es = slice(ec * 512, (ec + 1) * 512)
                if ec == 0:
                    stC["ysb"] = yst.tile([128, 2, 512], F32, tag="y",
                                          name="y_sb")
                y_sb = stC["ysb"]
                ps_y = psX.tile([128, 512], F32, tag="x", name="ps_y")
                h1l = stC.get("h1_last")
                if tt >= 12 and h1l is not None:
                    for p in range(PAIRS - 1):
                        nc.tensor.matmul(ps_y, h_t[p][:, ts_],
                                         wo_sb[:, p, es],
                                         start=(p == 0), stop=False)
                    nc.tensor.matmul(ps_y, h_t[3][0:64, ts_],
                                     wo_sb[0:64, 3, es],
                                     start=False, stop=False)
                    hs = slice(tt * 128 - 1536, tt * 128 - 1536 + 128)
                    nc.tensor.matmul(ps_y, h1l[0:64, hs], wo2_sb[:, es],
                                     start=False, stop=True)
                else:
                    for p in range(PAIRS):
                        nc.tensor.matmul(ps_y, h_t[p][:, ts_],
                                         wo_sb[:, p, es],
                                         start=(p == 0),
                                         stop=(p == PAIRS - 1))
                # keep the out-proj evacuations off ACT: it is saturated
                # with exps in these windows, and evac jitter there stalls
                # the score-PSUM ring
                if ec == 0:
                    nc.vector.tensor_copy(y_sb[:, 0, :], ps_y)
                    if tt >= 12:
                        # per-half stores at the very end shorten the drain
                        nc.sync.dma_start(y[ts_, 0:512], y_sb[:, 0, :])
                else:
                    nc.vector.tensor_copy(y_sb[:, 1, :], ps_y)
                    if tt >= 12:
                        nc.sync.dma_start(y[ts_, 512:1024], y_sb[:, 1, :])
                    else:
                        # one full-row DMA per t-tile
                        nc.sync.dma_start(y[ts_, :], y_sb)

            def emit_pv(st):
                p, ps_h, e_sb, c0, k, first, last = st
                nc.tensor.matmul(ps_h[0:65, 0, c0:],
                                 v_sb[:, k, 130 * p:130 * p + 65],
                                 e_sb[:, 0, c0:],
                                 start=first, stop=last)
                nc.tensor.matmul(ps_h[0:65, 1, c0:],
                                 v_sb[:, k, 130 * p + 65:130 * p + 130],
                                 e_sb[:, 1, c0:],
                                 start=first, stop=last)

            def emit_norm_head(p, ps_h, qc, h1, last=False):
                # evacuate H + sums to SBUF in one copy: ps_h (the single
                # PSUM PV buffer) frees after just this op, so the next
                # q-chunk's first PV is not gated on the whole chain.  For
                # the very last q-chunk there is no next PV — skip the
                # staging copy and read PSUM directly (shorter chain).
                if last:
                    s_h = ps_h
                else:
                    s_h = shp.tile([65, 2, 512], F32, tag="sh", name="s_h")
                    nc.vector.tensor_copy(s_h, ps_h[0:65, :, :])
                # the custom-DVE reciprocal misreads rows at non-zero base
                # partitions on hardware — stage the sums row at partition 0
                s1 = sB.tile([1, 2, 512], F32, tag="s1", name="s1")
                nc.vector.tensor_copy(s1, s_h[64:65, :, :])
                r1 = sB.tile([1, 2, 512], F32, tag="r1", name="r1")
                nc.vector.reciprocal_approx_fast(out=r1, in_=s1)
                r64 = rbp.tile([64, 2, 512], F32, tag="r64", name="r64")
                nc.gpsimd.partition_broadcast(r64, r1)
                return (p, qc, h1, s_h, r64)

            def emit_norm_tail(pend):
                # deferred: the DVE queue only sees these multiplies once
                # the Pool broadcast has surely landed (strict FIFOs — a
                # waiting instruction blocks everything behind it)
                p, qc, h1, s_h, r64 = pend
                qs = slice(qc * QC, (qc + 1) * QC)
                nc.vector.tensor_mul(h_t[p][0:64, qs], s_h[0:64, 0, :],
                                     r64[0:64, 0, :])
                nc.vector.tensor_mul(h1[0:64, :], s_h[0:64, 1, :],
                                     r64[0:64, 1, :])
                # odd head rows into partitions 64:128 of the pair tile
                nc.sync.dma_start(h_t[p][64:128, qs], h1[0:64, :])

            pend_pv = []
            pend_norm = None
            feed = []
            for p in range(PAIRS):
                if p < PAIRS - 1:
                    feed = [lambda pp=p + 1, kti=kti: v_unit(pp, kti)
                            for kti in range(NKT)]
                qt, kt = qt_t[p], kt_t[p]
                for qc in range(NQC):
                    nk = 4 * (qc + 1)
                    ps_h = psH.tile([128, 2, 512], F32, tag="pv", name="ps_h")
                    h1 = h1p.tile([64, QC], BF16, tag="h1", name="h1")
                    # process the diagonal k-tiles FIRST: their
                    # affine_selects (Pool) then hide behind the rest of
                    # the window instead of gating the end-of-chunk drain
                    k_order = list(range(4 * qc, nk)) + list(range(4 * qc))
                    for ki, k in enumerate(k_order):
                        m = k - 4 * qc
                        c0 = 128 * m if m >= 0 else 0
                        qs = slice(qc * QC + c0, (qc + 1) * QC)
                        ks = slice(k * KT, (k + 1) * KT)
                        ps_s = psS.tile([128, 2, 512], F32, tag="s",
                                        name="ps_s")
                        diag = m >= 0
                        nc.tensor.matmul(ps_s[:, 0, c0:], kt[0:64, ks],
                                         qt[0:64, qs], start=True,
                                         stop=True)
                        nc.tensor.matmul(ps_s[:, 1, c0:], kt[64:128, ks],
                                         qt[64:128, qs], start=True,
                                         stop=True)
                        e_sb = ep.tile([128, 2, 512], BF16, tag="e",
                                       name="e_sb")
                        nc.scalar.activation(e_sb[:, :, c0:], ps_s[:, :, c0:],
                                             AF.Exp, scale=0.125)
                        if diag:
                            # zero the strictly-upper-tri (k>q) entries of
                            # the diagonal block after exp
                            nc.gpsimd.affine_select(
                                e_sb[:, :, c0:c0 + 128],
                                e_sb[:, :, c0:c0 + 128],
                                pattern=[[0, 2], [1, 128]],
                                compare_op=ALU.is_ge,
                                fill=0.0, base=0, channel_multiplier=-1)
                        if len(pend_pv) >= 4:
                            emit_pv(pend_pv.pop(0))
                        pend_pv.append((p, ps_h, e_sb, c0, k,
                                        ki == 0, ki == nk - 1))
                        # fire the deferred norm tail only once the Pool
                        # broadcast had time to land
                        if pend_norm is not None and ki >= 3:
                            emit_norm_tail(pend_norm)
                            pend_norm = None
                        # PE filler: V projections of the next pair (or,
                        # for the last pair, out-projection tiles; ki>=5 so
                        # they follow this q-chunk's norm tail)
                        if p < PAIRS - 1:
                            if feed and ki % 5 in (1, 3):
                                feed.pop(0)()
                        elif ki >= 5:
                            if feed:
                                feed.pop(0)()
                            if feed and ki % 2 == 0:
                                feed.pop(0)()
                    # cover the last exp's latency, then drain the PVs and
                    # normalize this q-chunk
                    if feed and p < PAIRS - 1:
                        feed.pop(0)()
                    while pend_pv:
                        emit_pv(pend_pv.pop(0))
                    if pend_norm is not None:
                        emit_norm_tail(pend_norm)
                    if p == PAIRS - 1 and qc == NQC - 1:
                        stC["h1_last"] = h1
                    pend_norm = emit_norm_head(
                        p, ps_h, qc, h1,
                        last=(p == PAIRS - 1 and qc == NQC - 1))
                    if p == PAIRS - 1:
                        for tt in range(4 * qc, 4 * qc + 4):
                            feed.append(lambda tt=tt: c_unit(tt, 0))
                            feed.append(lambda tt=tt: c_unit(tt, 1))
                if p < PAIRS - 1:
                    # the next pair's attention needs its V complete
                    while feed:
                        feed.pop(0)()
            emit_norm_tail(pend_norm)
            # tail: remaining out-projection tiles
            while feed:
                feed.pop(0)()

        qkv_stack.close()

    nc.compile()
    return nc


def _host_prep(in_features, token_positions, Wq, Wk, Wv, Wo):
    """Shard + pre-transpose + bf16-cast inputs for the 8 cores."""
    x = np.asarray(in_features, dtype=np.float32)
    pos = np.asarray(token_positions)
    Wq = np.asarray(Wq, dtype=np.float32)
    Wk = np.asarray(Wk, dtype=np.float32)
    Wv = np.asarray(Wv, dtype=np.float32)
    Wo = np.asarray(Wo, dtype=np.float32)

    # rotate-half permutation of each head's 64 dims: evens then odds
    perm = np.concatenate([np.arange(0, 64, 2), np.arange(1, 64, 2)])
    full_perm = (np.arange(NUM_HEADS)[:, None] * 64 + perm[None, :]).reshape(-1)
    Wq_p = Wq[full_perm, :]   # permute output rows (head dims)
    Wk_p = Wk[full_perm, :]

    # P2: alt = P2 @ pre (per 64-block: alt[i] = -pre[32+i], alt[32+i]=pre[i])
    p2 = np.zeros((128, 128), np.float32)
    for blk in (0, 64):
        for i in range(32):
            p2[blk + i, blk + 32 + i] = -1.0
            p2[blk + 32 + i, blk + i] = 1.0
    p2t = np.ascontiguousarray(p2.T).astype(ml_dtypes.bfloat16)

    inv_freq = 1.0 / (THETA ** (np.arange(32, dtype=np.float64) * 2.0 / 64))

    bf = ml_dtypes.bfloat16
    in_maps = []
    for core in range(N_CORES):
        b = core // 2
        g = core % 2
        hs = slice(g * 512, (g + 1) * 512)   # head-dim slice of d_model

        ang = pos[b].astype(np.float64)[None, :] * inv_freq[:, None]  # [32,T]
        cos64 = np.cos(ang).astype(np.float32)
        sin64 = np.sin(ang).astype(np.float32)
        cos128 = np.tile(np.concatenate([cos64, cos64], 0), (2, 1))   # [128,T]
        sin128 = np.tile(np.concatenate([sin64, sin64], 0), (2, 1))

        in_maps.append({
            "xt": np.ascontiguousarray(x[b].T).astype(bf),
            "wq": np.ascontiguousarray(Wq_p[hs, :].T).astype(bf),
            "wk": np.ascontiguousarray(Wk_p[hs, :].T).astype(bf),
            "wv": np.ascontiguousarray(Wv[hs, :].T).astype(bf),
            "wo": np.ascontiguousarray(Wo[:, hs].T).astype(bf),
            "cos": np.ascontiguousarray(cos128).astype(bf),
            "sin": np.ascontiguousarray(sin128).astype(bf),
            "p2t": p2t,
        })
    return in_maps


def kernel(**inputs):
    if "nc" not in _CACHE:
        _CACHE["nc"] = _build_nc()
    nc = _CACHE["nc"]
    in_maps = _host_prep(**inputs)
    res = run_bass_kernel_spmd(nc, in_maps, core_ids=list(range(N_CORES)))
    out = np.zeros((B, T, D_MODEL), np.float32)
    for core in range(N_CORES):
        out[core // 2] += res.results[core]["y"]
    return out


# revision 28
# speedup vs baseline: 1.0764x; 1.0116x over previous
"""Causal multi-head self-attention (RoPE) Trainium2 Bass kernel.

Problem: B=4, T=2048, d_model=1024, 16 heads, d_k=64, causal, RoPE,
fp32 I/O.  Sharding: batch (4-way) x head-group (2-way) over 8 cores.
Core c handles batch c//2 and heads [8*(c%2) .. 8*(c%2)+8).

Everything on device runs in the "transposed" domain to avoid on-chip
transposes entirely:
  QT/KT:  [head_dim, T]   (head dim on partitions), bf16
  V:      [T, head_dim]   (k positions on partitions), bf16 + ones col
  scores: S^T [k, q] = KT_tile^T @ QT  (per head), fp32 PSUM
  E = exp(S^T/8) in bf16; causal masking of the diagonal blocks via
  gpsimd affine_select (zero out k>q entries after exp)
  PV: H^T[d, q] = [V|ones]^T @ E  -> heads rows 0:64 + sums row 64
  normalize via fast reciprocal of the sums row + gpsimd
  partition_broadcast + DVE multiplies
  out-proj: y[t, e] = H^T_tile^T @ WoT, accumulated over head pairs

All matmuls run in bf16 (1 cycle/row at any moving size); accumulation
is fp32 in PSUM.  RoPE is applied as rot = cos (.) pre + sin (.)
(P2 @ pre), where the head dim has been host-permuted to rotate-half
layout and P2 is the fixed swap-negate permutation folded into a
128x128 matmul.

The k-loop and projection loops are software-pipelined by one stage so
the PE never stalls on ACT (exp / psum evacuation) latency.
"""

import numpy as np
from contextlib import ExitStack

import concourse.bass as bass
from concourse import bacc
import concourse.tile as tile
import concourse.mybir as mybir
import ml_dtypes
from concourse.bass_utils import run_bass_kernel_spmd

F32 = mybir.dt.float32
BF16 = mybir.dt.bfloat16
AF = mybir.ActivationFunctionType
ALU = mybir.AluOpType

D_MODEL = 1024
NUM_HEADS = 16
THETA = 10000.0
B, T = 4, 2048
N_CORES = 8
PAIRS = 4             # head pairs per core (8 heads)
QC = 512              # q-chunk width
NQC = T // QC
KT = 128              # k-tile height
NKT = T // KT
XC = 256              # xt streaming chunk width (phase A)
NXC = T // XC
DMT = D_MODEL // 128  # 8 d_model k-tiles

_CACHE = {}


def _build_nc():
    nc = bacc.Bacc(None, target_bir_lowering=False)

    xt = nc.dram_tensor("xt", [D_MODEL, T], BF16, kind="ExternalInput")
    wq = nc.dram_tensor("wq", [D_MODEL, 512], BF16, kind="ExternalInput")
    wk = nc.dram_tensor("wk", [D_MODEL, 512], BF16, kind="ExternalInput")
    wv = nc.dram_tensor("wv", [D_MODEL, 512], BF16, kind="ExternalInput")
    wo = nc.dram_tensor("wo", [512, D_MODEL], BF16, kind="ExternalInput")
    cosd = nc.dram_tensor("cos", [128, T], BF16, kind="ExternalInput")
    sind = nc.dram_tensor("sin", [128, T], BF16, kind="ExternalInput")
    p2t = nc.dram_tensor("p2t", [128, 128], BF16, kind="ExternalInput")
    y = nc.dram_tensor("y", [T, D_MODEL], F32, kind="ExternalOutput")

    with tile.TileContext(nc) as tc, ExitStack() as ctx:
        constp = ctx.enter_context(tc.tile_pool(name="const", bufs=1))
        qkv_stack = ExitStack()
        qkp = qkv_stack.enter_context(tc.tile_pool(name="qk", bufs=1))
        vp = qkv_stack.enter_context(tc.tile_pool(name="v", bufs=1))

        cos_sb = constp.tile([128, T], BF16)
        sin_sb = constp.tile([128, T], BF16)
        p2_sb = constp.tile([128, 128], BF16)
        # wo lives in a long-lived pool so it can be prefetched in phase B
        wo_sb = constp.tile([128, PAIRS, D_MODEL], BF16)
        # pair-3 odd-head rows of wo staged at partitions 0:64 so the final
        # out-proj tiles can read the h1 tile directly (skipping the h1->h_t
        # DMA on the tail critical path)
        wo2_sb = constp.tile([64, D_MODEL], BF16)
        # 1x64 ones row for the final PE-broadcast of the softmax sums
        F32R = mybir.dt.float32r
        ones1_sb = constp.tile([1, 64], F32R)
        nc.vector.memset(ones1_sb.bitcast(F32), 1.0)

        qt_t = [qkp.tile([128, T], BF16, name=f"qt{p}", tag=f"qt{p}")
                for p in range(PAIRS)]
        kt_t = [qkp.tile([128, T], BF16, name=f"kt{p}", tag=f"kt{p}")
                for p in range(PAIRS)]
        # V with interleaved ones cols: per k-tile, per pair:
        # [V_h0(64) | 1 | V_h1(64) | 1] = 130 cols
        v_sb = vp.tile([128, NKT, PAIRS * 130], BF16)
        v5 = v_sb.rearrange("p t (pr x) -> p t pr x", pr=PAIRS)
        v6 = v5.rearrange("p t pr (hl c) -> p t pr hl c", hl=2)
        nc.vector.memset(v6[:, :, :, :, 64:65], 1.0)

        # x and wv stay resident through phase B: the V projections of
        # pairs 1-3 are interleaved into the attention windows (which are
        # Activation-bound) as pure-PE filler work
        xp = qkv_stack.enter_context(tc.tile_pool(name="xp", bufs=1))
        wvp = qkv_stack.enter_context(tc.tile_pool(name="wvp", bufs=1))
        x_sb = xp.tile([128, DMT, T], BF16)
        wv_sb = wvp.tile([128, DMT, 512], BF16)
        psX = qkv_stack.enter_context(tc.tile_pool(name="psX", bufs=2,
                                                   space="PSUM"))
        yst = qkv_stack.enter_context(tc.tile_pool(name="yst", bufs=3))
        hp = qkv_stack.enter_context(
            tc.tile_pool(name="hp", bufs=1, side="right"))
        h_t = [hp.tile([128, T], BF16, name=f"h{p}", tag=f"h{p}")
               for p in range(PAIRS)]

        def v_unit(pp, kti):
            ps_x = psX.tile([128, 512], F32, tag="x", name="ps_v")
            ps_v = ps_x[:, 0:128]
            for dmt in range(DMT):
                nc.tensor.matmul(
                    ps_v, x_sb[:, dmt, kti * 128:(kti + 1) * 128],
                    wv_sb[:, dmt, pp * 128:(pp + 1) * 128],
                    start=(dmt == 0), stop=(dmt == DMT - 1))
            src = ps_v.rearrange("p (hl c) -> p hl c", hl=2)
            nc.vector.tensor_copy(v6[:, kti, pp, :, 0:64], src)

        # ---------------- Phase A: projections + RoPE ----------------
        with tc.tile_pool(name="wqkv", bufs=1) as wp, \
             tc.tile_pool(name="sA", bufs=5) as sA, \
             tc.tile_pool(name="psA", bufs=3, space="PSUM") as psA:
            wq_sb = wp.tile([128, DMT, 512], BF16)
            wk_sb = wp.tile([128, DMT, 512], BF16)
            xt_r = xt.rearrange("(a p) t -> p a t", p=128)
            wq_r = wq.rearrange("(a p) m -> p a m", p=128)
            wk_r = wk.rearrange("(a p) m -> p a m", p=128)
            wv_r = wv.rearrange("(a p) m -> p a m", p=128)
            # priority order: interleave wq + first x chunk per-dmt so the
            # first matmul group can start as soon as its slices land;
            # everything else trickles in behind them.
            # consolidated prefix: every DMA pays ~625ns of HWDGE, so
            # fewer, larger transfers get wk/cos/sin on chip sooner
            nc.sync.dma_start(wq_sb[:, 0:4, :], wq_r[:, 0:4, :])
            nc.sync.dma_start(x_sb[:, 0:4, 0:XC], xt_r[:, 0:4, 0:XC])
            nc.sync.dma_start(wq_sb[:, 4:8, :], wq_r[:, 4:8, :])
            nc.sync.dma_start(x_sb[:, 4:8, 0:XC], xt_r[:, 4:8, 0:XC])
            nc.sync.dma_start(p2_sb, p2t[:])
            nc.sync.dma_start(cos_sb, cosd[:])
            nc.sync.dma_start(sin_sb, sind[:])
            nc.sync.dma_start(wk_sb[:, 0:4, :], wk_r[:, 0:4, :])
            nc.sync.dma_start(wk_sb[:, 4:8, :], wk_r[:, 4:8, :])
            nc.sync.dma_start(x_sb[:, :, XC:2 * XC], xt_r[:, :, XC:2 * XC])
            # wv is needed by the first V unit at the start of chunk 1
            nc.sync.dma_start(wv_sb[:, 0:4, :], wv_r[:, 0:4, :])
            nc.sync.dma_start(wv_sb[:, 4:8, :], wv_r[:, 4:8, :])
            for c in range(2, NXC):
                cs = slice(c * XC, (c + 1) * XC)
                nc.sync.dma_start(x_sb[:, :, cs], xt_r[:, :, cs])

            def rope_stage(st):
                pre, dst, cs = st
                ps_a = psA.tile([128, XC], F32, tag="alt", name="ps_a")
                nc.tensor.matmul(ps_a, p2_sb, pre, start=True, stop=True)
                t1 = sA.tile([128, XC], F32, tag="t1", name="t1")
                nc.vector.tensor_mul(t1, sin_sb[:, cs], ps_a)
                t2 = sA.tile([128, XC], F32, tag="t2", name="t2")
                nc.gpsimd.tensor_mul(t2, cos_sb[:, cs], pre)
                nc.vector.tensor_add(dst[:, cs], t1, t2)

            for c in range(NXC):
                cs = slice(c * XC, (c + 1) * XC)
                pend = None
                if c > 0:
                    # pair-0 V for the previous chunk (deferred one chunk
                    # so the wv DMA has time to land)
                    v_unit(0, 2 * (c - 1))
                    v_unit(0, 2 * (c - 1) + 1)
                for (w_sb, dsts) in ((wq_sb, qt_t), (wk_sb, kt_t)):
                    for p in range(PAIRS):
                        ps = psA.tile([128, XC], F32, tag="proj", name="ps")
                        for dmt in range(DMT):
                            nc.tensor.matmul(
                                ps, w_sb[:, dmt, p * 128:(p + 1) * 128],
                                x_sb[:, dmt, cs],
                                start=(dmt == 0), stop=(dmt == DMT - 1))
                        pre = sA.tile([128, XC], BF16, tag="pre", name="pre")
                        nc.scalar.copy(pre, ps)
                        if pend is not None:
                            rope_stage(pend)
                        pend = (pre, dsts[p], cs)

                rope_stage(pend)
            v_unit(0, 14)
            v_unit(0, 15)

        # ---------------- Phase B: attention ----------------
        with tc.tile_pool(name="ep", bufs=7) as ep, \
             tc.tile_pool(name="h1p", bufs=2) as h1p, \
             tc.tile_pool(name="sB", bufs=3) as sB, \
             tc.tile_pool(name="shp", bufs=2) as shp, \
             tc.tile_pool(name="rbp", bufs=3) as rbp, \
             tc.tile_pool(name="psS", bufs=2, space="PSUM") as psS, \
             tc.tile_pool(name="psH", bufs=1, space="PSUM") as psH:

            # prefetch the out-projection weights while the PE chews on
            # attention — the DMA engines are nearly idle in phase B
            nc.sync.dma_start(
                wo_sb, wo.rearrange("(a p) m -> p a m", p=128))
            nc.sync.dma_start(wo2_sb, wo[448:512, :])

            stC = {"ysb": None}

            def c_unit(tt, ec):
                ts_ = slice(tt * 128, (tt + 1) * 128)
                es = slice(ec * 512, (ec + 1) * 512)
                if ec == 0:
                    stC["ysb"] = yst.tile([128, 2, 512], F32, tag="y",
                                          name="y_sb")
                y_sb = stC["ysb"]
                ps_y = psX.tile([128, 512], F32, tag="x", name="ps_y")
                h1l = stC.get("h1_last")
                if tt >= 12 and h1l is not None:
                    for p in range(PAIRS - 1):
                        nc.tensor.matmul(ps_y, h_t[p][:, ts_],
                                         wo_sb[:, p, es],
                                         start=(p == 0), stop=False)
                    nc.tensor.matmul(ps_y, h_t[3][0:64, ts_],
                                     wo_sb[0:64, 3, es],
                                     start=False, stop=False)
                    hs = slice(tt * 128 - 1536, tt * 128 - 1536 + 128)
                    nc.tensor.matmul(ps_y, h1l[0:64, hs], wo2_sb[:, es],
                                     start=False, stop=True)
                else:
                    for p in range(PAIRS):
                        nc.tensor.matmul(ps_y, h_t[p][:, ts_],
                                         wo_sb[:, p, es],
                                         start=(p == 0),
                                         stop=(p == PAIRS - 1))
                # keep the out-proj evacuations off ACT: it is saturated
                # with exps in these windows, and evac jitter there stalls
                # the score-PSUM ring
                if ec == 0:
                    nc.vector.tensor_copy(y_sb[:, 0, :], ps_y)
                    if tt >= 12:
                        # per-half stores at the very end shorten the drain
                        nc.sync.dma_start(y[ts_, 0:512], y_sb[:, 0, :])
                else:
                    nc.vector.tensor_copy(y_sb[:, 1, :], ps_y)
                    if tt >= 12:
                        nc.sync.dma_start(y[ts_, 512:1024], y_sb[:, 1, :])
                    else:
                        # one full-row DMA per t-tile
                        nc.sync.dma_start(y[ts_, :], y_sb)

            def emit_pv(st):
                p, ps_h, e_sb, c0, k, first, last = st
                nc.tensor.matmul(ps_h[0:65, 0, c0:],
                                 v_sb[:, k, 130 * p:130 * p + 65],
                                 e_sb[:, 0, c0:],
                                 start=first, stop=last)
                nc.tensor.matmul(ps_h[0:65, 1, c0:],
                                 v_sb[:, k, 130 * p + 65:130 * p + 130],
                                 e_sb[:, 1, c0:],
                                 start=first, stop=last)

            def emit_norm_head(p, ps_h, qc, h1, last=False):
                # evacuate H + sums to SBUF in one copy: ps_h (the single
                # PSUM PV buffer) frees after just this op, so the next
                # q-chunk's first PV is not gated on the whole chain.  For
                # the very last q-chunk there is no next PV — skip the
                # staging copy and read PSUM directly (shorter chain).
                if last:
                    s_h = ps_h
                else:
                    s_h = shp.tile([65, 2, 512], F32, tag="sh", name="s_h")
                    nc.vector.tensor_copy(s_h, ps_h[0:65, :, :])
                # the custom-DVE reciprocal misreads rows at non-zero base
                # partitions on hardware — stage the sums row at partition 0
                s1 = sB.tile([1, 2, 512], F32, tag="s1", name="s1")
                nc.vector.tensor_copy(s1, s_h[64:65, :, :])
                if last:
                    # this chain gates the final out-proj tiles and the PE
                    # is idle here: broadcast the sums via a ones-matmul
                    # into free score PSUM, then take the reciprocal at
                    # base partition 0 (both hardware-proven paths)
                    ps_r = psS.tile([128, 2, 512], F32, tag="s", name="ps_r")
                    s1r = s1.bitcast(F32R)
                    nc.tensor.matmul(ps_r[0:64, 0, :], ones1_sb,
                                     s1r[0:1, 0, :], start=True, stop=True)
                    nc.tensor.matmul(ps_r[0:64, 1, :], ones1_sb,
                                     s1r[0:1, 1, :], start=True, stop=True)
                    r64 = rbp.tile([64, 2, 512], F32, tag="r64", name="r64")
                    nc.vector.reciprocal_approx_fast(out=r64[0:64, 0, :],
                                                     in_=ps_r[0:64, 0, :])
                    nc.vector.reciprocal_approx_fast(out=r64[0:64, 1, :],
                                                     in_=ps_r[0:64, 1, :])
                    return (p, qc, h1, s_h, r64)
                r1 = sB.tile([1, 2, 512], F32, tag="r1", name="r1")
                nc.vector.reciprocal_approx_fast(out=r1, in_=s1)
                r64 = rbp.tile([64, 2, 512], F32, tag="r64", name="r64")
                nc.gpsimd.partition_broadcast(r64, r1)
                return (p, qc, h1, s_h, r64)

            def emit_norm_tail(pend):
                # deferred: the DVE queue only sees these multiplies once
                # the Pool broadcast has surely landed (strict FIFOs — a
                # waiting instruction blocks everything behind it)
                p, qc, h1, s_h, r64 = pend
                qs = slice(qc * QC, (qc + 1) * QC)
                nc.vector.tensor_mul(h_t[p][0:64, qs], s_h[0:64, 0, :],
                                     r64[0:64, 0, :])
                nc.vector.tensor_mul(h1[0:64, :], s_h[0:64, 1, :],
                                     r64[0:64, 1, :])
                # odd head rows into partitions 64:128 of the pair tile
                nc.sync.dma_start(h_t[p][64:128, qs], h1[0:64, :])

            pend_pv = []
            pend_norm = None
            feed = []
            for p in range(PAIRS):
                if p < PAIRS - 1:
                    feed = [lambda pp=p + 1, kti=kti: v_unit(pp, kti)
                            for kti in range(NKT)]
                qt, kt = qt_t[p], kt_t[p]
                for qc in range(NQC):
                    nk = 4 * (qc + 1)
                    ps_h = psH.tile([128, 2, 512], F32, tag="pv", name="ps_h")
                    h1 = h1p.tile([64, QC], BF16, tag="h1", name="h1")
                    # process the diagonal k-tiles FIRST: their
                    # affine_selects (Pool) then hide behind the rest of
                    # the window instead of gating the end-of-chunk drain
                    k_order = list(range(4 * qc, nk)) + list(range(4 * qc))
                    for ki, k in enumerate(k_order):
                        m = k - 4 * qc
                        c0 = 128 * m if m >= 0 else 0
                        qs = slice(qc * QC + c0, (qc + 1) * QC)
                        ks = slice(k * KT, (k + 1) * KT)
                        ps_s = psS.tile([128, 2, 512], F32, tag="s",
                                        name="ps_s")
                        diag = m >= 0
                        nc.tensor.matmul(ps_s[:, 0, c0:], kt[0:64, ks],
                                         qt[0:64, qs], start=True,
                                         stop=True)
                        nc.tensor.matmul(ps_s[:, 1, c0:], kt[64:128, ks],
                                         qt[64:128, qs], start=True,
                                         stop=True)
                        e_sb = ep.tile([128, 2, 512], BF16, tag="e",
                                       name="e_sb")
                        nc.scalar.activation(e_sb[:, :, c0:], ps_s[:, :, c0:],
                                             AF.Exp, scale=0.125)
                        if diag:
                            # zero the strictly-upper-tri (k>q) entries of
                            # the diagonal block after exp
                            nc.gpsimd.affine_select(
                                e_sb[:, :, c0:c0 + 128],
                                e_sb[:, :, c0:c0 + 128],
                                pattern=[[0, 2], [1, 128]],
                                compare_op=ALU.is_ge,
                                fill=0.0, base=0, channel_multiplier=-1)
                        if len(pend_pv) >= 4:
                            emit_pv(pend_pv.pop(0))
                        pend_pv.append((p, ps_h, e_sb, c0, k,
                                        ki == 0, ki == nk - 1))
                        # fire the deferred norm tail only once the Pool
                        # broadcast had time to land
                        if pend_norm is not None and ki >= 3:
                            emit_norm_tail(pend_norm)
                            pend_norm = None
                        # PE filler: V projections of the next pair (or,
                        # for the last pair, out-projection tiles; ki>=5 so
                        # they follow this q-chunk's norm tail)
                        if p < PAIRS - 1:
                            if feed and ki % 5 in (1, 3):
                                feed.pop(0)()
                        elif ki >= 5:
                            if feed:
                                feed.pop(0)()
                            if feed and ki % 2 == 0:
                                feed.pop(0)()
                    # cover the last exp's latency, then drain the PVs and
                    # normalize this q-chunk
                    if feed and p < PAIRS - 1:
                        feed.pop(0)()
                    while pend_pv:
                        emit_pv(pend_pv.pop(0))
                    if pend_norm is not None:
                        emit_norm_tail(pend_norm)
                    if p == PAIRS - 1 and qc == NQC - 1:
                        stC["h1_last"] = h1
                    pend_norm = emit_norm_head(
                        p, ps_h, qc, h1,
                        last=(p == PAIRS - 1 and qc == NQC - 1))
                    if p == PAIRS - 1:
                        for tt in range(4 * qc, 4 * qc + 4):
                            feed.append(lambda tt=tt: c_unit(tt, 0))
                            feed.append(lambda tt=tt: c_unit(tt, 1))
                if p < PAIRS - 1:
                    # the next pair's attention needs its V complete
                    while feed:
                        feed.pop(0)()
            emit_norm_tail(pend_norm)
            # tail: remaining out-projection tiles
            while feed:
                feed.pop(0)()

        qkv_stack.close()

    nc.compile()
    return nc


def _host_prep(in_features, token_positions, Wq, Wk, Wv, Wo):
    """Shard + pre-transpose + bf16-cast inputs for the 8 cores."""
    x = np.asarray(in_features, dtype=np.float32)
    pos = np.asarray(token_positions)
    Wq = np.asarray(Wq, dtype=np.float32)
    Wk = np.asarray(Wk, dtype=np.float32)
    Wv = np.asarray(Wv, dtype=np.float32)
    Wo = np.asarray(Wo, dtype=np.float32)

    # rotate-half permutation of each head's 64 dims: evens then odds
    perm = np.concatenate([np.arange(0, 64, 2), np.arange(1, 64, 2)])
    full_perm = (np.arange(NUM_HEADS)[:, None] * 64 + perm[None, :]).reshape(-1)
    Wq_p = Wq[full_perm, :]   # permute output rows (head dims)
    Wk_p = Wk[full_perm, :]

    # P2: alt = P2 @ pre (per 64-block: alt[i] = -pre[32+i], alt[32+i]=pre[i])
    p2 = np.zeros((128, 128), np.float32)
    for blk in (0, 64):
        for i in range(32):
            p2[blk + i, blk + 32 + i] = -1.0
            p2[blk + 32 + i, blk + i] = 1.0
    p2t = np.ascontiguousarray(p2.T).astype(ml_dtypes.bfloat16)

    inv_freq = 1.0 / (THETA ** (np.arange(32, dtype=np.float64) * 2.0 / 64))

    bf = ml_dtypes.bfloat16
    in_maps = []
    for core in range(N_CORES):
        b = core // 2
        g = core % 2
        hs = slice(g * 512, (g + 1) * 512)   # head-dim slice of d_model

        ang = pos[b].astype(np.float64)[None, :] * inv_freq[:, None]  # [32,T]
        cos64 = np.cos(ang).astype(np.float32)
        sin64 = np.sin(ang).astype(np.float32)
        cos128 = np.tile(np.concatenate([cos64, cos64], 0), (2, 1))   # [128,T]
        sin128 = np.tile(np.concatenate([sin64, sin64], 0), (2, 1))

        in_maps.append({
            "xt": np.ascontiguousarray(x[b].T).astype(bf),
            "wq": np.ascontiguousarray(Wq_p[hs, :].T).astype(bf),
            "wk": np.ascontiguousarray(Wk_p[hs, :].T).astype(bf),
            "wv": np.ascontiguousarray(Wv[hs, :].T).astype(bf),
            "wo": np.ascontiguousarray(Wo[:, hs].T).astype(bf),
            "cos": np.ascontiguousarray(cos128).astype(bf),
            "sin": np.ascontiguousarray(sin128).astype(bf),
            "p2t": p2t,
        })
    return in_maps


def kernel(**inputs):
    if "nc" not in _CACHE:
        _CACHE["nc"] = _build_nc()
    nc = _CACHE["nc"]
    in_maps = _host_prep(**inputs)
    res = run_bass_kernel_spmd(nc, in_maps, core_ids=list(range(N_CORES)))
    out = np.zeros((B, T, D_MODEL), np.float32)
    for core in range(N_CORES):
        out[core // 2] += res.results[core]["y"]
    return out
